# revision 25
# baseline (speedup 1.0000x reference)
"""Trainium2 Bass kernel for BoxMultiHeadedAttention (B=8, N=512, D=512, H=8).

Sharding: data-parallel over batch — each of the 8 NeuronCores computes one
batch element end-to-end; weights replicated; no collectives.

Per-core algorithm (transposed-attention layout [m(part), n(free)]):
  * q/k/v projections on PE (bf16) from PE-transposed inputs; PSUM evictions
    on ACT (Identity with scale/bias folds the q/k biases; 1/8 folded into k).
  * geometry wg:
      - dx/dy: ln fields via ACT Square(bias=-c)/Ln + DVE sub/clamp; phase
        fractions t = (alpha_j/4pi)*dx2 by f32r selector matmuls on PE;
        magic-round fold on DVE (rr, ff) + |f|-1/4 on Pool; ONE stacked ACT
        Sin pass yields [sin(2pi f); -cos(2pi f)] with the cos sign folded
        into the WBLK weights; WG contraction on PE (bf16).
      - dw/dh: exactly separable -> rank-64 PE contraction of per-box
        sin/cos banks.
      - h-major -> m-major partition permutation via 8 merged strided DMAs
        per row-block (3-dim APs).
  * exp-domain softmax: T = E*(1 + obj*wgd); E on ACT, obj-mult on Pool,
    wgd-mult/add on DVE; row sums via PE one-hot matmul; 1/s broadcast via
    PE selector matmul (f32r); final linear on PE.
  * bv is folded into bo on the host (bo' = bo + bv @ Wo).
"""
import math
import numpy as np
from contextlib import ExitStack

import concourse.bass as bass
import concourse.mybir as mybir
import concourse.tile as tile
from concourse.bass_utils import run_bass_kernel_spmd

F32 = mybir.dt.float32
F32R = mybir.dt.float32r
BF16 = mybir.dt.bfloat16
AF = mybir.ActivationFunctionType
ALU = mybir.AluOpType

B, N, D, H = 8, 512, 512, 8
DK = D // H
P = 128
NRB = N // P
NG = 8
GM = 16
WAVE_LEN = 1000.0
MAGIC = 12582912.0
C2 = float(2.0 * math.log(0.001))
ESHIFT = -6.0
TWO_PI = float(2.0 * math.pi)

_alphas = (100.0 / (WAVE_LEN ** (np.arange(8) / 8.0))).astype(np.float64)

# const-blob column offsets (f32 blob)
OFF_IDENT = 0          # [P, 128]
OFF_OH8 = 128          # [P, 64]  col = h*8 + c
OFF_WBLK = 192         # [P, 4*128]
OFF_W1E = 704          # [64, 128]
OFF_BG = 832           # [P, 1]
OFF_ACOL = 833         # [64, 1]
OFF_PCOLM = 834        # [64, 1]
OFF_PCOLN = 835        # [64, 1]
OFF_REPL = 836         # [8, 4*128]  col = ob*128 + p; 1 iff h == ob*2+p//64
CBLOB_W = 1348
# f32r blob: selap [P, 4*128] (col q*128+c)
CBLOBR_W = 512


def _split_multi_waits(nc):
    """walrus here accepts only ONE sync-wait per ISA instruction; hoist
    extras onto NoOps inserted before the offending instruction."""
    n_fix = 0
    for blk in nc.main_func.blocks:
        insts = list(blk.instructions)
        out, dirty = [], False
        for inst in insts:
            si = inst.sync_info
            waits = list(si.on_wait) if si is not None else []
            if len(waits) > 1:
                for kk, w in enumerate(waits[:-1]):
                    out.append(mybir.InstNoOp(
                        name=f"I-waitfix-{n_fix}-{kk}", engine=inst.engine,
                        sync_info=mybir.SyncInfo(on_wait=[w], on_update=[])))
                inst.sync_info = mybir.SyncInfo(
                    on_wait=[waits[-1]], on_update=list(si.on_update))
                n_fix += 1
                dirty = True
            out.append(inst)
        if dirty:
            blk.instructions = out
    return n_fix


def _build_cblob(WG, bG):
    cb = np.zeros((P, CBLOB_W), dtype=np.float32)
    cb[:, OFF_IDENT:OFF_IDENT + P] = np.eye(P, dtype=np.float32)
    # one-hot columns for row sums: OH8[p, h*8+c] = 1 iff c == h
    for h in range(H):
        cb[:, OFF_OH8 + h * H + h] = 1.0
    # WBLK: direct sin/cos weights.  sin4 layout: [:,0,:]=sin dx,
    # [:,1,:]=sin dy, [:,2,:]=cos dx, [:,3,:]=cos dy.
    gmap = [lambda j: j, lambda j: 8 + j, lambda j: 32 + j, lambda j: 40 + j]
    gscl = [1.0, 1.0, 1.0, 1.0]
    for c in range(4):
        for ml in range(GM):
            for j in range(8):
                for h in range(H):
                    cb[ml * 8 + j, OFF_WBLK + c * P + h * GM + ml] = \
                        gscl[c] * WG[h, gmap[c](j)]
    # dw/dh rank-64 weights (angle-addition banks; unchanged from the
    # half-angle formulation since the banks encode sin/cos via phase
    # offsets in PCOL)
    w1 = np.zeros((64, H), np.float32)
    acol = np.zeros((64,), np.float32)
    pcol_m = np.zeros((64,), np.float32)
    pcol_n = np.zeros((64,), np.float32)
    for f in range(2):
        for j in range(8):
            gs = 16 + 8 * f + j
            gc = 48 + 8 * f + j
            a = _alphas[j] / (4.0 * math.pi)
            for t in range(4):
                k = (f * 8 + j) * 4 + t
                acol[k] = a
                pcol_m[k] = 0.25 if t in (0, 2) else 0.0
                if t == 0:
                    pcol_n[k] = 0.0; w1[k] = WG[:, gs]
                elif t == 1:
                    pcol_n[k] = 0.75; w1[k] = WG[:, gs]   # -cos -> +pi
                elif t == 2:
                    pcol_n[k] = 0.25; w1[k] = WG[:, gc]
                else:
                    pcol_n[k] = 0.0; w1[k] = WG[:, gc]
    cb[0:64, OFF_W1E:OFF_W1E + P] = np.repeat(w1, GM, axis=1)
    cb[:, OFF_BG] = np.repeat(bG.astype(np.float64), GM).astype(np.float32)
    cb[0:64, OFF_ACOL] = acol
    cb[0:64, OFF_PCOLM] = pcol_m
    cb[0:64, OFF_PCOLN] = pcol_n
    for ob in range(H // 2):
        for hi in range(2):
            cb[ob * 2 + hi, OFF_REPL + ob * P + hi * DK:
               OFF_REPL + ob * P + (hi + 1) * DK] = 1.0
    return cb


def _build_cblobr():
    cr = np.zeros((P, CBLOBR_W), dtype=np.float32)
    # SELAP[64*W + q*16 + ml, q*128 + ml*8 + j] = alpha_j/(4pi)
    for W in range(2):
        for q in range(4):
            for ml in range(GM):
                for j in range(8):
                    cr[64 * W + q * GM + ml, q * P + ml * 8 + j] = \
                        _alphas[j] / (4.0 * math.pi)
    return cr


def _host_prep(inputs):
    q = np.asarray(inputs["input_query"], np.float32)
    k = np.asarray(inputs["input_key"], np.float32)
    v = np.asarray(inputs["input_value"], np.float32)
    box = np.asarray(inputs["input_box"], np.float32)
    mask = np.asarray(inputs["mask"])
    nobj = np.asarray(inputs["not_objects"])
    WG = np.asarray(inputs["WG"], np.float32)
    bG = np.asarray(inputs["bG"], np.float32)
    Wo = np.asarray(inputs["Wo"], np.float32)
    bo = np.asarray(inputs["bo"], np.float32)
    bv = np.asarray(inputs["bv"], np.float32)

    x_min, y_min, x_max, y_max = [box[..., i] for i in range(4)]
    cx = (x_min + x_max) * 0.5
    cy = (y_min + y_max) * 0.5
    ww = x_max - x_min + 1.0
    hh = y_max - y_min + 1.0
    l2w = (2.0 * np.log(ww)).astype(np.float32)
    l2h = (2.0 * np.log(hh)).astype(np.float32)

    maskcol = (np.where(mask == 0, -1e9, 0.0) + ESHIFT).astype(np.float32)
    obj = (1.0 - nobj.astype(np.float32)).astype(np.float32)
    borow = (bo.astype(np.float64) + bv.astype(np.float64)
             @ Wo.astype(np.float64)).astype(np.float32)

    def col(a):  # [N] -> [P, NRB]
        return a.reshape(NRB, P).T

    shared = {
        "Wq": np.asarray(inputs["Wq"], np.float32),
        "Wk": np.asarray(inputs["Wk"], np.float32),
        "Wv": np.asarray(inputs["Wv"], np.float32),
        "Wo": Wo,
        "CBLOB": _build_cblob(WG, bG),
        "CBLOBR": _build_cblobr(),
    }
    bqc = col(np.asarray(inputs["bq"], np.float32))
    bkc = col(np.asarray(inputs["bk"], np.float32))
    in_maps = []
    for b in range(B):
        cols = np.zeros((P, 28), np.float32)
        for ob in range(4):
            cols[:, 24 + ob] = 1.0
            cols[2 * ob, 24 + ob] = 0.0
            cols[2 * ob + 1, 24 + ob] = 0.0
        cols[:, 0:4] = col(maskcol[b])
        cols[:, 4:8] = bqc
        cols[:, 8:12] = bkc
        cols[:, 12:16] = -col(cx[b])
        cols[:, 16:20] = -col(cy[b])
        cols[:, 20:24] = col(obj[b])
        rows = np.stack([cx[b], cy[b], l2w[b], l2h[b], obj[b], borow], 0)
        m = dict(shared)
        m.update({
            "xq": q[b].copy(), "xk": k[b].copy(), "xv": v[b].copy(),
            "COLS": cols, "ROWS": rows.astype(np.float32).copy(),
        })
        in_maps.append(m)
    return in_maps


def build_nc(fix_waits=True, perm_merge=True, debug=False):
    nc = bass.Bass()

    def dp(name, shape, dt=F32):
        return nc.declare_dram_parameter(name, list(shape), dt, isOutput=False)

    xq = dp("xq", (N, D)); xk = dp("xk", (N, D)); xv = dp("xv", (N, D))
    Wq = dp("Wq", (D, D)); Wk = dp("Wk", (D, D)); Wv = dp("Wv", (D, D))
    Wo = dp("Wo", (D, D))
    CBLOB = dp("CBLOB", (P, CBLOB_W))
    CBLOBR = dp("CBLOBR", (P, CBLOBR_W), F32R)
    COLS = dp("COLS", (P, 28))
    ROWS = dp("ROWS", (6, N))
    out = nc.declare_dram_parameter("out", [N, D], F32, isOutput=True)
    wgd_dram = nc.dram_tensor("wgd_scratch", [NRB, H, P, N], BF16)

    with ExitStack() as ctx:
        tc = ctx.enter_context(tile.TileContext(nc))
        const = ctx.enter_context(tc.tile_pool(name="const", bufs=1))
        persist = ctx.enter_context(tc.tile_pool(name="persist", bufs=1))

        # ---- const loads (DMA order favors phase-1/2 start) ----
        xq_sb = persist.tile([P, NRB, D], F32, tag="xq_sb")
        nc.sync.dma_start(xq_sb[:], xq.rearrange("(rb p) d -> p rb d", p=P))
        cF = const.tile([P, CBLOB_W], F32, tag="cF")
        nc.sync.dma_start(cF[:], CBLOB[:])
        cols_t = const.tile([P, 28], F32, tag="cols")
        nc.sync.dma_start(cols_t[:], COLS[:])
        rows_t = const.tile([P, 6, N], F32, tag="rows")
        nc.sync.dma_start(rows_t[:], ROWS[None, :, :].to_broadcast((P, 6, N)))
        wq_f = persist.tile([P, NRB, D], F32, tag="wq_f")
        nc.sync.dma_start(wq_f[:], Wq.rearrange("(kb p) d -> p kb d", p=P))
        xk_sb = persist.tile([P, NRB, D], F32, tag="xk_sb")
        nc.sync.dma_start(xk_sb[:], xk.rearrange("(rb p) d -> p rb d", p=P))
        xv_sb = persist.tile([P, NRB, D], F32, tag="xv_sb")
        nc.sync.dma_start(xv_sb[:], xv.rearrange("(rb p) d -> p rb d", p=P))
        wk_f = persist.tile([P, NRB, D], F32, tag="wk_f")
        nc.sync.dma_start(wk_f[:], Wk.rearrange("(kb p) d -> p kb d", p=P))
        wv_f = persist.tile([P, NRB, D], F32, tag="wv_f")
        nc.sync.dma_start(wv_f[:], Wv.rearrange("(kb p) d -> p kb d", p=P))
        cR = const.tile([P, CBLOBR_W], F32R, tag="cR")
        nc.sync.dma_start(cR[:], CBLOBR[:])
        wo_f = persist.tile([P, NRB, D], F32, tag="wo_f")
        nc.sync.dma_start(wo_f[:], Wo.rearrange("(kb p) d -> p kb d", p=P))

        ident = cF[:, OFF_IDENT:OFF_IDENT + P]
        mcol = cols_t[:, 0:4]
        bqcol = cols_t[:, 4:8]
        bkcol = cols_t[:, 8:12]
        negcx = cols_t[:, 12:16]
        negcy = cols_t[:, 16:20]
        ocol = cols_t[:, 20:24]
        zcol = cols_t[:, 24:28]
        cxbc = rows_t[:, 0, :]
        cybc = rows_t[:, 1, :]
        l2wbc = rows_t[:, 2, :]
        l2hbc = rows_t[:, 3, :]
        objbc_f = rows_t[:, 4, :]
        bobc = rows_t[:, 5, :]

        # small const casts / derived
        oh8_b = const.tile([P, H * H], BF16, tag="oh8b")
        nc.vector.tensor_copy(oh8_b[:], cF[:, OFF_OH8:OFF_OH8 + H * H])
        wblk_b = const.tile([P, 4, P], BF16, tag="wblkb")
        for c in range(4):
            nc.gpsimd.tensor_copy(wblk_b[:, c, :],
                                  cF[:, OFF_WBLK + c * P:OFF_WBLK + (c + 1) * P])
        w1e_b = const.tile([64, P], BF16, tag="w1eb")
        nc.gpsimd.tensor_copy(w1e_b[:], cF[0:64, OFF_W1E:OFF_W1E + P])
        objbc = const.tile([P, N], BF16, tag="objbc")
        nc.gpsimd.tensor_copy(objbc[:], objbc_f[:])
        halfpi = const.tile([P, 1], F32, tag="halfpi")
        nc.vector.memset(halfpi[:], float(math.pi / 2.0))
        bgm1 = const.tile([P, 1], F32, tag="bgm1")
        nc.vector.tensor_scalar(bgm1[:], cF[:, OFF_BG:OFF_BG + 1], -1.0, None,
                                ALU.add)
        acol = cF[0:64, OFF_ACOL:OFF_ACOL + 1]
        pcolm = cF[0:64, OFF_PCOLM:OFF_PCOLM + 1]
        pcoln = cF[0:64, OFF_PCOLN:OFF_PCOLN + 1]

        # ---------------- phases 1+2 (shared scope so they overlap) -------
        dxy2 = persist.tile([P, NRB, 2, N], F32R, tag="dxy2")
        qT = persist.tile([P, NRB, N], BF16, tag="qT")
        kTt = persist.tile([P, NRB, N], BF16, tag="kT")
        v_sb = persist.tile([P, NRB, D], BF16, tag="v_sb")
        wo_b = persist.tile([P, NRB, D], BF16, tag="wob")
        ot = persist.tile([P, NRB, N], BF16, tag="ot")

        with tc.tile_pool(name="tpool", bufs=1) as tpool, \
             tc.tile_pool(name="work2", bufs=3) as work2, \
             tc.tile_pool(name="work1", bufs=2) as work1, \
             tc.tile_pool(name="psum1", bufs=3, space="PSUM") as psum1:
            # phase 2: ln fields (ACT Square/Ln + DVE sub/clamp)
            for rb in range(NRB):
                for (ci, cbc, ncol, l2bc) in ((0, cxbc, negcx, l2wbc),
                                              (1, cybc, negcy, l2hbc)):
                    d2 = work2.tile([P, N], F32, tag="geo_d2")
                    nc.scalar.activation(d2[:], cbc, AF.Square,
                                         bias=ncol[:, rb:rb + 1])
                    l2t = work2.tile([P, N], F32, tag="geo_l2")
                    nc.scalar.activation(l2t[:], d2[:], AF.Ln)
                    g_ = work2.tile([P, N], F32, tag="geo_g")
                    nc.vector.tensor_tensor(g_[:], l2t[:], l2bc, ALU.subtract)
                    nc.vector.tensor_scalar_max(dxy2[:, rb, ci, :], g_[:], C2)

            # phase 1: transposes + projections
            xqTb = tpool.tile([P, NRB, N], BF16, tag="xqTb")
            xkTb = tpool.tile([P, NRB, N], BF16, tag="xkTb")
            xvTb = tpool.tile([P, NRB, N], BF16, tag="xvTb")
            wq_b = tpool.tile([P, NRB, D], BF16, tag="wqb")
            wk_b = tpool.tile([P, NRB, D], BF16, tag="wkb")
            wv_b = tpool.tile([P, NRB, D], BF16, tag="wvb")

            kk = 0
            for (xs, dstb) in ((xq_sb, xqTb), (xk_sb, xkTb), (xv_sb, xvTb)):
                for rb in range(NRB):
                    for cb in range(NRB):
                        tp = psum1.tile([P, P], F32, tag="tp")
                        nc.tensor.transpose(tp[:], xs[:, rb, cb * P:(cb + 1) * P],
                                            ident)
                        dst = dstb[:, cb, rb * P:(rb + 1) * P]
                        if kk % 2 == 0:
                            nc.vector.tensor_copy(dst, tp[:])
                        else:
                            nc.scalar.activation(dst, tp[:], AF.Identity)
                        kk += 1
            for (wf, wb_) in ((wq_f, wq_b), (wk_f, wk_b), (wv_f, wv_b),
                              (wo_f, wo_b)):
                nc.gpsimd.tensor_copy(wb_[:], wf[:])

            for (wb_, xb, dstT, bcol, scl) in (
                    (wq_b, xqTb, qT, bqcol, 1.0),
                    (wk_b, xkTb, kTt, bkcol, 0.125)):
                for ob in range(NRB):
                    ps = psum1.tile([P, N], F32, tag="projps")
                    for kb in range(NRB):
                        nc.tensor.matmul(ps[:],
                                         wb_[:, kb, ob * P:(ob + 1) * P],
                                         xb[:, kb, :],
                                         start=(kb == 0),
                                         stop=(kb == NRB - 1))
                    nc.scalar.activation(dstT[:, ob, :], ps[:], AF.Identity,
                                         scale=scl, bias=bcol[:, ob:ob + 1])
            for mb in range(NRB):
                ps = psum1.tile([P, D], F32, tag="projps")
                for kb in range(NRB):
                    nc.tensor.matmul(ps[:], xvTb[:, kb, mb * P:(mb + 1) * P],
                                     wv_b[:, kb, :],
                                     start=(kb == 0), stop=(kb == NRB - 1))
                nc.scalar.activation(v_sb[:, mb, :], ps[:], AF.Identity)

        # ---------------- phase 3: dw/dh banks ----------------
        bankM = persist.tile([64, N], BF16, tag="bankM")
        bankN = persist.tile([64, N], BF16, tag="bankN")
        with tc.tile_pool(name="work3", bufs=2) as work3:
            for (pcol, bank) in ((pcolm, bankM), (pcoln, bankN)):
                t_ = work3.tile([64, N], F32, tag="bk_t")
                nc.vector.tensor_scalar(t_[:32, :], l2wbc[:32, :],
                                        acol[:32, :], pcol[:32, :],
                                        ALU.mult, ALU.add)
                nc.vector.tensor_scalar(t_[32:, :], l2hbc[32:64, :],
                                        acol[32:, :], pcol[32:, :],
                                        ALU.mult, ALU.add)
                r_ = work3.tile([64, N], F32, tag="bk_r")
                nc.vector.tensor_scalar(r_[:], t_[:], MAGIC, -MAGIC,
                                        ALU.add, ALU.add)
                f_ = work3.tile([64, N], F32, tag="bk_f")
                nc.vector.tensor_tensor(f_[:], t_[:], r_[:], ALU.subtract)
                nc.scalar.activation(bank[:], f_[:], AF.Sin, scale=TWO_PI)

        # ---------------- phase 4: wg ----------------
        wgdT = persist.tile([P, H, NRB, N], BF16, tag="wgdT")
        with tc.tile_pool(name="work4", bufs=2) as work4, \
             tc.tile_pool(name="psum_u", bufs=2, space="PSUM") as psum_u, \
             tc.tile_pool(name="psum_wg", bufs=2, space="PSUM") as psum_wg:
            for rb in range(NRB):
                wgd_il = work4.tile([P, NG, N], BF16, tag="wgd_il")
                for g in range(NG):
                    off = 64 * (g // 4)
                    qq = g % 4
                    ups = psum_u.tile([P, 2, N], F32, tag="ups")
                    for ci in range(2):
                        nc.tensor.matmul(ups[:, ci, :],
                                         cR[off:off + 64, qq * P:(qq + 1) * P],
                                         dxy2[off:off + 64, rb, ci, :],
                                         start=True, stop=True)
                    rr = work4.tile([P, 2, N], F32, tag="fold_r")
                    nc.vector.tensor_scalar(rr[:], ups[:], MAGIC, -MAGIC,
                                            ALU.add, ALU.add)
                    ff = work4.tile([P, 2, N], F32, tag="fold_f")
                    nc.vector.tensor_tensor(ff[:], ups[:], rr[:],
                                            ALU.subtract)
                    habs = work4.tile([P, 2, N], F32, tag="habs")
                    nc.scalar.activation(habs[:], ff[:], AF.Abs)
                    sin4 = work4.tile([P, 4, N], BF16, tag="sin4")
                    nc.scalar.activation(sin4[:, 0:2, :], ff[:], AF.Sin,
                                         scale=TWO_PI)
                    # cos(2pi f) = sin(pi/2 - 2pi |f|)
                    nc.scalar.activation(sin4[:, 2:4, :], habs[:], AF.Sin,
                                         scale=-TWO_PI, bias=halfpi[:])
                    lhs_wh = work4.tile([64, P], BF16, tag="lhs_wh")
                    mbase = rb * P + g * GM
                    nc.gpsimd.tensor_tensor(
                        lhs_wh[:].rearrange("k (h m) -> k h m", h=H),
                        w1e_b[:].rearrange("k (h m) -> k h m", h=H),
                        bankM[:, mbase:mbase + GM][:, None, :]
                            .to_broadcast((64, H, GM)),
                        ALU.mult)
                    wgp = psum_wg.tile([P, N], F32, tag="wgp")
                    for c in range(4):
                        nc.tensor.matmul(wgp[:], wblk_b[:, c, :],
                                         sin4[:, c, :],
                                         start=(c == 0), stop=False)
                    nc.tensor.matmul(wgp[:], lhs_wh[:], bankN[:],
                                     start=False, stop=True)
                    # wgd = max(wg + bG, 1e-6) - 1 = max(wg + (bG-1), 1e-6-1)
                    nc.vector.tensor_scalar(wgd_il[:, g, :], wgp[:],
                                            bgm1[:], 1e-6 - 1.0,
                                            ALU.add, ALU.max)
                # h-major -> m-major permutation via DRAM bounce
                # (SBUF->SBUF DMA honors only one partition dim on HW, and
                # SBUF-side APs may carry only one partition dim, so the
                # write side goes per (rb, h)).
                for h in range(H):
                    nc.sync.dma_start(
                        wgd_dram[rb, h]
                            .rearrange("(g ml) n -> ml g n", g=NG),
                        wgd_il[h * GM:(h + 1) * GM, :, :])
                nc.sync.dma_start(
                    wgdT[:, :, rb, :],
                    wgd_dram[rb].rearrange("h p n -> p h n"))

        # ---------------- phase 5: attention ----------------
        with tc.tile_pool(name="work5", bufs=3) as work5, \
             tc.tile_pool(name="psum5", bufs=3, space="PSUM") as psum5, \
             tc.tile_pool(name="psum_s", bufs=1, space="PSUM") as psum_s, \
             tc.tile_pool(name="psum_av", bufs=2, space="PSUM") as psum_av, \
             tc.tile_pool(name="psum_rb", bufs=1, space="PSUM") as psum_rb, \
             tc.tile_pool(name="dbgpool", bufs=1) as dbgpool:

            objpair = persist.tile([P, NRB, N], BF16, tag="objpair")
            for rb in range(NRB):
                nc.vector.tensor_scalar(objpair[:, rb, :], objbc[:],
                                        ocol[:, rb:rb + 1], None, ALU.mult)
            # head PAIRS (2k, 2k+1) share kT/qT block ob=k at offsets 0/64
            for ob in range(H // 2):
                h0 = 2 * ob
                av = psum_av.tile([P, N], F32, tag="avps")
                sbank = psum_s.tile([H, N], F32, tag="sbank")
                for rb in range(NRB):
                    for hi in range(2):
                        po = hi * DK
                        h = h0 + hi
                        st = psum5.tile([P, N], F32, tag="stps")
                        nc.tensor.matmul(
                            st[:],
                            kTt[po:po + DK, ob, rb * P:(rb + 1) * P],
                            qT[po:po + DK, ob, :], start=True, stop=True)
                        e_ = work5.tile([P, N], BF16, tag="e_t")
                        nc.scalar.activation(e_[:], st[:], AF.Exp,
                                             bias=mcol[:, rb:rb + 1])
                        e1 = work5.tile([P, N], BF16, tag="e1_t")
                        e1_eng = (nc.gpsimd if (2 * rb + hi) % 2 == 0
                                  else nc.vector)
                        e1_eng.tensor_tensor(e1[:], e_[:], objpair[:, rb, :],
                                             ALU.mult)
                        e2 = work5.tile([P, N], BF16, tag="e2_t")
                        nc.vector.tensor_tensor(e2[:], e1[:],
                                                wgdT[:, h, rb, :],
                                                ALU.mult)
                        tt_ = work5.tile([P, N], BF16, tag="tt_t")
                        nc.vector.tensor_tensor(tt_[:], e_[:], e2[:], ALU.add)
                        nc.tensor.matmul(sbank[:],
                                         oh8_b[:, h * H:(h + 1) * H],
                                         tt_[:],
                                         start=(rb == 0 and hi == 0),
                                         stop=(rb == NRB - 1 and hi == 1),
                                         skip_group_check=True)
                        nc.tensor.matmul(av[po:po + DK, :],
                                         v_sb[:, rb,
                                              h * DK:(h + 1) * DK],
                                         tt_[:], start=(rb == 0),
                                         stop=(rb == NRB - 1),
                                         skip_group_check=True)
                if debug and ob == 0:
                    dbg_sb = nc.declare_dram_parameter(
                        "dbg_sbank", [H, N], F32, isOutput=True)
                    sb_c = dbgpool.tile([H, N], F32, tag="dbg_sbc")
                    nc.vector.tensor_copy(sb_c[:], sbank[:])
                    nc.sync.dma_start(dbg_sb[:], sb_c[:])
                    dbg_av = nc.declare_dram_parameter(
                        "dbg_av", [P, N], F32, isOutput=True)
                    av_c = dbgpool.tile([P, N], F32, tag="dbg_avc")
                    nc.vector.tensor_copy(av_c[:], av[:])
                    nc.sync.dma_start(dbg_av[:], av_c[:])
                sb2 = work5.tile([H, N], F32, tag="sb2")
                nc.vector.tensor_scalar(sb2[:], sbank[:],
                                        zcol[0:8, ob:ob + 1], None, ALU.add)
                rs = work5.tile([H, N], F32, tag="rs")
                nc.vector.reciprocal(rs[:], sb2[:])
                rrb = psum_rb.tile([P, N], F32, tag="rrb")
                nc.tensor.matmul(rrb[:], cF[0:8, OFF_REPL + ob * P:
                                            OFF_REPL + (ob + 1) * P],
                                 rs[0:8, :], start=True, stop=True)
                if debug and ob == 0:
                    dbg_rs = nc.declare_dram_parameter(
                        "dbg_rs", [H, N], F32, isOutput=True)
                    nc.sync.dma_start(dbg_rs[:], rs[:])
                    dbg_rrb = nc.declare_dram_parameter(
                        "dbg_rrb", [P, N], F32, isOutput=True)
                    rrb_c = dbgpool.tile([P, N], F32, tag="dbg_rrbc")
                    nc.vector.tensor_copy(rrb_c[:], rrb[:])
                    nc.sync.dma_start(dbg_rrb[:], rrb_c[:])
                rrb_sb = work5.tile([P, N], F32, tag="rrb_sb")
                nc.scalar.activation(rrb_sb[:], rrb[:], AF.Identity)
                nc.vector.tensor_tensor(ot[:, ob, :], av[:], rrb_sb[:],
                                        ALU.mult)

        if debug:
            dbg_dxy2 = nc.declare_dram_parameter(
                "dbg_dxy2", [P, NRB, 2, N], F32R, isOutput=True)
            nc.sync.dma_start(dbg_dxy2[:], dxy2[:])
            dbg_wgdT = nc.declare_dram_parameter(
                "dbg_wgdT", [P, H, NRB, N], BF16, isOutput=True)
            nc.sync.dma_start(dbg_wgdT[:], wgdT[:])
            dbg_qT = nc.declare_dram_parameter(
                "dbg_qT", [P, NRB, N], BF16, isOutput=True)
            nc.sync.dma_start(dbg_qT[:], qT[:])
            dbg_kT = nc.declare_dram_parameter(
                "dbg_kT", [P, NRB, N], BF16, isOutput=True)
            nc.sync.dma_start(dbg_kT[:], kTt[:])
            dbg_v = nc.declare_dram_parameter(
                "dbg_v", [P, NRB, D], BF16, isOutput=True)
            nc.sync.dma_start(dbg_v[:], v_sb[:])
            dbg_ot = nc.declare_dram_parameter(
                "dbg_ot", [P, NRB, N], BF16, isOutput=True)
            nc.sync.dma_start(dbg_ot[:], ot[:])

        # final projection: out[n, d]
        with tc.tile_pool(name="work6", bufs=2) as work6, \
             tc.tile_pool(name="psum6", bufs=2, space="PSUM") as psum6:
            for r in range(NRB):
                ps = psum6.tile([P, D], F32, tag="fps")
                for kt in range(NRB):
                    nc.tensor.matmul(ps[:], ot[:, kt, r * P:(r + 1) * P],
                                     wo_b[:, kt, :],
                                     start=(kt == 0), stop=(kt == NRB - 1))
                fo = work6.tile([P, D], F32, tag="fo")
                nc.vector.tensor_tensor(fo[:], ps[:], bobc, ALU.add)
                nc.sync.dma_start(out[r * P:(r + 1) * P, :], fo[:])

    if fix_waits:
        _split_multi_waits(nc)
    return nc


_NC_CACHE = {}


def kernel(**inputs):
    in_maps = _host_prep(inputs)
    if "nc" not in _NC_CACHE:
        _NC_CACHE["nc"] = build_nc()
    nc = _NC_CACHE["nc"]
    res = run_bass_kernel_spmd(nc, in_maps, list(range(B)))
    out = np.stack([res.results[b]["out"] for b in range(B)], axis=0)
    return out.astype(np.float32)


if __name__ == "__main__":
    print("kernel module ok")


# revision 26
# speedup vs baseline: 1.0257x; 1.0257x over previous
"""Trainium2 Bass kernel for BoxMultiHeadedAttention (B=8, N=512, D=512, H=8).

Sharding: data-parallel over batch — each of the 8 NeuronCores computes one
batch element end-to-end; weights replicated; no collectives.

Per-core algorithm (transposed-attention layout [m(part), n(free)]):
  * q/k/v projections on PE (bf16) from PE-transposed inputs; PSUM evictions
    on ACT (Identity with scale/bias folds the q/k biases; 1/8 folded into k).
  * geometry wg:
      - dx/dy: ln fields via ACT Square(bias=-c)/Ln + DVE sub/clamp; phase
        fractions t = (alpha_j/4pi)*dx2 by f32r selector matmuls on PE;
        magic-round fold on DVE (rr, ff) + |f|-1/4 on Pool; ONE stacked ACT
        Sin pass yields [sin(2pi f); -cos(2pi f)] with the cos sign folded
        into the WBLK weights; WG contraction on PE (bf16).
      - dw/dh: exactly separable -> rank-64 PE contraction of per-box
        sin/cos banks.
      - h-major -> m-major partition permutation via 8 merged strided DMAs
        per row-block (3-dim APs).
  * exp-domain softmax: T = E*(1 + obj*wgd); E on ACT, obj-mult on Pool,
    wgd-mult/add on DVE; row sums via PE one-hot matmul; 1/s broadcast via
    PE selector matmul (f32r); final linear on PE.
  * bv is folded into bo on the host (bo' = bo + bv @ Wo).
"""
import math
import numpy as np
from contextlib import ExitStack

import concourse.bass as bass
import concourse.mybir as mybir
import concourse.tile as tile
from concourse.bass_utils import run_bass_kernel_spmd

F32 = mybir.dt.float32
F32R = mybir.dt.float32r
BF16 = mybir.dt.bfloat16
AF = mybir.ActivationFunctionType
ALU = mybir.AluOpType

B, N, D, H = 8, 512, 512, 8
DK = D // H
P = 128
NRB = N // P
NG = 8
GM = 16
WAVE_LEN = 1000.0
MAGIC = 12582912.0
C2 = float(2.0 * math.log(0.001))
ESHIFT = -6.0
TWO_PI = float(2.0 * math.pi)

_alphas = (100.0 / (WAVE_LEN ** (np.arange(8) / 8.0))).astype(np.float64)

# const-blob column offsets (f32 blob)
OFF_IDENT = 0          # [P, 128]
OFF_OH8 = 128          # [P, 64]  col = h*8 + c
OFF_WBLK = 192         # [P, 4*128]
OFF_W1E = 704          # [64, 128]
OFF_BG = 832           # [P, 1]
OFF_ACOL = 833         # [64, 1]
OFF_PCOLM = 834        # [64, 1]
OFF_PCOLN = 835        # [64, 1]
OFF_REPL = 836         # [8, 4*128]  col = ob*128 + p; 1 iff h == ob*2+p//64
CBLOB_W = 1348
# f32r blob: selap [P, 4*128] (col q*128+c)
CBLOBR_W = 512


def _split_multi_waits(nc):
    """walrus here accepts only ONE sync-wait per ISA instruction; hoist
    extras onto NoOps inserted before the offending instruction."""
    n_fix = 0
    for blk in nc.main_func.blocks:
        insts = list(blk.instructions)
        out, dirty = [], False
        for inst in insts:
            si = inst.sync_info
            waits = list(si.on_wait) if si is not None else []
            if len(waits) > 1:
                for kk, w in enumerate(waits[:-1]):
                    out.append(mybir.InstNoOp(
                        name=f"I-waitfix-{n_fix}-{kk}", engine=inst.engine,
                        sync_info=mybir.SyncInfo(on_wait=[w], on_update=[])))
                inst.sync_info = mybir.SyncInfo(
                    on_wait=[waits[-1]], on_update=list(si.on_update))
                n_fix += 1
                dirty = True
            out.append(inst)
        if dirty:
            blk.instructions = out
    return n_fix


def _build_cblob(WG, bG):
    cb = np.zeros((P, CBLOB_W), dtype=np.float32)
    cb[:, OFF_IDENT:OFF_IDENT + P] = np.eye(P, dtype=np.float32)
    # one-hot columns for row sums: OH8[p, h*8+c] = 1 iff c == h
    for h in range(H):
        cb[:, OFF_OH8 + h * H + h] = 1.0
    # WBLK: direct sin/cos weights.  sin4 layout: [:,0,:]=sin dx,
    # [:,1,:]=sin dy, [:,2,:]=cos dx, [:,3,:]=cos dy.
    gmap = [lambda j: j, lambda j: 8 + j, lambda j: 32 + j, lambda j: 40 + j]
    gscl = [1.0, 1.0, 1.0, 1.0]
    for c in range(4):
        for ml in range(GM):
            for j in range(8):
                for h in range(H):
                    cb[ml * 8 + j, OFF_WBLK + c * P + h * GM + ml] = \
                        gscl[c] * WG[h, gmap[c](j)]
    # dw/dh rank-64 weights (angle-addition banks; unchanged from the
    # half-angle formulation since the banks encode sin/cos via phase
    # offsets in PCOL)
    w1 = np.zeros((64, H), np.float32)
    acol = np.zeros((64,), np.float32)
    pcol_m = np.zeros((64,), np.float32)
    pcol_n = np.zeros((64,), np.float32)
    for f in range(2):
        for j in range(8):
            gs = 16 + 8 * f + j
            gc = 48 + 8 * f + j
            a = _alphas[j] / (4.0 * math.pi)
            for t in range(4):
                k = (f * 8 + j) * 4 + t
                acol[k] = a
                pcol_m[k] = 0.25 if t in (0, 2) else 0.0
                if t == 0:
                    pcol_n[k] = 0.0; w1[k] = WG[:, gs]
                elif t == 1:
                    pcol_n[k] = 0.75; w1[k] = WG[:, gs]   # -cos -> +pi
                elif t == 2:
                    pcol_n[k] = 0.25; w1[k] = WG[:, gc]
                else:
                    pcol_n[k] = 0.0; w1[k] = WG[:, gc]
    cb[0:64, OFF_W1E:OFF_W1E + P] = np.repeat(w1, GM, axis=1)
    cb[:, OFF_BG] = np.repeat(bG.astype(np.float64), GM).astype(np.float32)
    cb[0:64, OFF_ACOL] = acol
    cb[0:64, OFF_PCOLM] = pcol_m
    cb[0:64, OFF_PCOLN] = pcol_n
    for ob in range(H // 2):
        for hi in range(2):
            cb[ob * 2 + hi, OFF_REPL + ob * P + hi * DK:
               OFF_REPL + ob * P + (hi + 1) * DK] = 1.0
    return cb


def _build_cblobr():
    cr = np.zeros((P, CBLOBR_W), dtype=np.float32)
    # SELAP[64*W + q*16 + ml, q*128 + ml*8 + j] = alpha_j/(4pi)
    for W in range(2):
        for q in range(4):
            for ml in range(GM):
                for j in range(8):
                    cr[64 * W + q * GM + ml, q * P + ml * 8 + j] = \
                        _alphas[j] / (4.0 * math.pi)
    return cr


def _host_prep(inputs):
    q = np.asarray(inputs["input_query"], np.float32)
    k = np.asarray(inputs["input_key"], np.float32)
    v = np.asarray(inputs["input_value"], np.float32)
    box = np.asarray(inputs["input_box"], np.float32)
    mask = np.asarray(inputs["mask"])
    nobj = np.asarray(inputs["not_objects"])
    WG = np.asarray(inputs["WG"], np.float32)
    bG = np.asarray(inputs["bG"], np.float32)
    Wo = np.asarray(inputs["Wo"], np.float32)
    bo = np.asarray(inputs["bo"], np.float32)
    bv = np.asarray(inputs["bv"], np.float32)

    x_min, y_min, x_max, y_max = [box[..., i] for i in range(4)]
    cx = (x_min + x_max) * 0.5
    cy = (y_min + y_max) * 0.5
    ww = x_max - x_min + 1.0
    hh = y_max - y_min + 1.0
    l2w = (2.0 * np.log(ww)).astype(np.float32)
    l2h = (2.0 * np.log(hh)).astype(np.float32)

    maskcol = (np.where(mask == 0, -1e9, 0.0) + ESHIFT).astype(np.float32)
    obj = (1.0 - nobj.astype(np.float32)).astype(np.float32)
    borow = (bo.astype(np.float64) + bv.astype(np.float64)
             @ Wo.astype(np.float64)).astype(np.float32)

    def col(a):  # [N] -> [P, NRB]
        return a.reshape(NRB, P).T

    shared = {
        "Wq": np.asarray(inputs["Wq"], np.float32),
        "Wk": np.asarray(inputs["Wk"], np.float32),
        "Wv": np.asarray(inputs["Wv"], np.float32),
        "Wo": Wo,
        "CBLOB": _build_cblob(WG, bG),
        "CBLOBR": _build_cblobr(),
    }
    bqc = col(np.asarray(inputs["bq"], np.float32))
    bkc = col(np.asarray(inputs["bk"], np.float32))
    in_maps = []
    for b in range(B):
        cols = np.zeros((P, 28), np.float32)
        for ob in range(4):
            cols[:, 24 + ob] = 1.0
            cols[2 * ob, 24 + ob] = 0.0
            cols[2 * ob + 1, 24 + ob] = 0.0
        cols[:, 0:4] = col(maskcol[b])
        cols[:, 4:8] = bqc
        cols[:, 8:12] = bkc
        cols[:, 12:16] = -col(cx[b])
        cols[:, 16:20] = -col(cy[b])
        cols[:, 20:24] = col(obj[b])
        rows = np.stack([cx[b], cy[b], l2w[b], l2h[b], obj[b], borow], 0)
        m = dict(shared)
        m.update({
            "xq": q[b].copy(), "xk": k[b].copy(), "xv": v[b].copy(),
            "COLS": cols, "ROWS": rows.astype(np.float32).copy(),
        })
        in_maps.append(m)
    return in_maps


def build_nc(fix_waits=True, perm_merge=True, debug=False):
    nc = bass.Bass()

    def dp(name, shape, dt=F32):
        return nc.declare_dram_parameter(name, list(shape), dt, isOutput=False)

    xq = dp("xq", (N, D)); xk = dp("xk", (N, D)); xv = dp("xv", (N, D))
    Wq = dp("Wq", (D, D)); Wk = dp("Wk", (D, D)); Wv = dp("Wv", (D, D))
    Wo = dp("Wo", (D, D))
    CBLOB = dp("CBLOB", (P, CBLOB_W))
    CBLOBR = dp("CBLOBR", (P, CBLOBR_W), F32R)
    COLS = dp("COLS", (P, 28))
    ROWS = dp("ROWS", (6, N))
    out = nc.declare_dram_parameter("out", [N, D], F32, isOutput=True)
    wgd_dram = nc.dram_tensor("wgd_scratch", [NRB, H, P, N], BF16)

    with ExitStack() as ctx:
        tc = ctx.enter_context(tile.TileContext(nc))
        const = ctx.enter_context(tc.tile_pool(name="const", bufs=1))
        persist = ctx.enter_context(tc.tile_pool(name="persist", bufs=1))

        # ---- const loads (DMA order favors phase-1/2 start) ----
        xq_sb = persist.tile([P, NRB, D], F32, tag="xq_sb")
        nc.sync.dma_start(xq_sb[:], xq.rearrange("(rb p) d -> p rb d", p=P))
        cF = const.tile([P, CBLOB_W], F32, tag="cF")
        nc.sync.dma_start(cF[:], CBLOB[:])
        cols_t = const.tile([P, 28], F32, tag="cols")
        nc.sync.dma_start(cols_t[:], COLS[:])
        rows_t = const.tile([P, 6, N], F32, tag="rows")
        nc.sync.dma_start(rows_t[:], ROWS[None, :, :].to_broadcast((P, 6, N)))
        wq_f = persist.tile([P, NRB, D], F32, tag="wq_f")
        nc.sync.dma_start(wq_f[:], Wq.rearrange("(kb p) d -> p kb d", p=P))
        xk_sb = persist.tile([P, NRB, D], F32, tag="xk_sb")
        nc.sync.dma_start(xk_sb[:], xk.rearrange("(rb p) d -> p rb d", p=P))
        xv_sb = persist.tile([P, NRB, D], F32, tag="xv_sb")
        nc.sync.dma_start(xv_sb[:], xv.rearrange("(rb p) d -> p rb d", p=P))
        wk_f = persist.tile([P, NRB, D], F32, tag="wk_f")
        nc.sync.dma_start(wk_f[:], Wk.rearrange("(kb p) d -> p kb d", p=P))
        wv_f = persist.tile([P, NRB, D], F32, tag="wv_f")
        nc.sync.dma_start(wv_f[:], Wv.rearrange("(kb p) d -> p kb d", p=P))
        cR = const.tile([P, CBLOBR_W], F32R, tag="cR")
        nc.sync.dma_start(cR[:], CBLOBR[:])
        wo_f = persist.tile([P, NRB, D], F32, tag="wo_f")
        nc.sync.dma_start(wo_f[:], Wo.rearrange("(kb p) d -> p kb d", p=P))

        ident = cF[:, OFF_IDENT:OFF_IDENT + P]
        mcol = cols_t[:, 0:4]
        bqcol = cols_t[:, 4:8]
        bkcol = cols_t[:, 8:12]
        negcx = cols_t[:, 12:16]
        negcy = cols_t[:, 16:20]
        ocol = cols_t[:, 20:24]
        zcol = cols_t[:, 24:28]
        cxbc = rows_t[:, 0, :]
        cybc = rows_t[:, 1, :]
        l2wbc = rows_t[:, 2, :]
        l2hbc = rows_t[:, 3, :]
        objbc_f = rows_t[:, 4, :]
        bobc = rows_t[:, 5, :]

        # small const casts / derived
        oh8_b = const.tile([P, H * H], BF16, tag="oh8b")
        nc.vector.tensor_copy(oh8_b[:], cF[:, OFF_OH8:OFF_OH8 + H * H])
        wblk_b = const.tile([P, 4, P], BF16, tag="wblkb")
        for c in range(4):
            nc.gpsimd.tensor_copy(wblk_b[:, c, :],
                                  cF[:, OFF_WBLK + c * P:OFF_WBLK + (c + 1) * P])
        w1e_b = const.tile([64, P], BF16, tag="w1eb")
        nc.gpsimd.tensor_copy(w1e_b[:], cF[0:64, OFF_W1E:OFF_W1E + P])
        objbc = const.tile([P, N], BF16, tag="objbc")
        nc.gpsimd.tensor_copy(objbc[:], objbc_f[:])
        halfpi = const.tile([P, 1], F32, tag="halfpi")
        nc.vector.memset(halfpi[:], float(math.pi / 2.0))
        bgm1 = const.tile([P, 1], F32, tag="bgm1")
        nc.vector.tensor_scalar(bgm1[:], cF[:, OFF_BG:OFF_BG + 1], -1.0, None,
                                ALU.add)
        acol = cF[0:64, OFF_ACOL:OFF_ACOL + 1]
        pcolm = cF[0:64, OFF_PCOLM:OFF_PCOLM + 1]
        pcoln = cF[0:64, OFF_PCOLN:OFF_PCOLN + 1]

        # ---------------- phases 1+2 (shared scope so they overlap) -------
        dxy2 = persist.tile([P, NRB, 2, N], F32R, tag="dxy2")
        qT = persist.tile([P, NRB, N], BF16, tag="qT")
        kTt = persist.tile([P, NRB, N], BF16, tag="kT")
        v_sb = persist.tile([P, NRB, D], BF16, tag="v_sb")
        wo_b = persist.tile([P, NRB, D], BF16, tag="wob")
        ot = persist.tile([P, NRB, N], BF16, tag="ot")

        with tc.tile_pool(name="tpool", bufs=1) as tpool, \
             tc.tile_pool(name="work2", bufs=3) as work2, \
             tc.tile_pool(name="work1", bufs=2) as work1, \
             tc.tile_pool(name="psum1", bufs=3, space="PSUM") as psum1:
            # phase 2: ln fields (ACT Square/Ln + DVE sub/clamp)
            for rb in range(NRB):
                for (ci, cbc, ncol, l2bc) in ((0, cxbc, negcx, l2wbc),
                                              (1, cybc, negcy, l2hbc)):
                    d2 = work2.tile([P, N], F32, tag="geo_d2")
                    nc.scalar.activation(d2[:], cbc, AF.Square,
                                         bias=ncol[:, rb:rb + 1])
                    l2t = work2.tile([P, N], F32, tag="geo_l2")
                    nc.scalar.activation(l2t[:], d2[:], AF.Ln)
                    g_ = work2.tile([P, N], F32, tag="geo_g")
                    nc.vector.tensor_tensor(g_[:], l2t[:], l2bc, ALU.subtract)
                    nc.vector.tensor_scalar_max(dxy2[:, rb, ci, :], g_[:], C2)

            # phase 1: transposes + projections
            xqTb = tpool.tile([P, NRB, N], BF16, tag="xqTb")
            xkTb = tpool.tile([P, NRB, N], BF16, tag="xkTb")
            xvTb = tpool.tile([P, NRB, N], BF16, tag="xvTb")
            wq_b = tpool.tile([P, NRB, D], BF16, tag="wqb")
            wk_b = tpool.tile([P, NRB, D], BF16, tag="wkb")
            wv_b = tpool.tile([P, NRB, D], BF16, tag="wvb")

            kk = 0
            for (xs, dstb) in ((xq_sb, xqTb), (xk_sb, xkTb), (xv_sb, xvTb)):
                for rb in range(NRB):
                    for cb in range(NRB):
                        tp = psum1.tile([P, P], F32, tag="tp")
                        nc.tensor.transpose(tp[:], xs[:, rb, cb * P:(cb + 1) * P],
                                            ident)
                        dst = dstb[:, cb, rb * P:(rb + 1) * P]
                        if kk % 2 == 0:
                            nc.vector.tensor_copy(dst, tp[:])
                        else:
                            nc.scalar.activation(dst, tp[:], AF.Identity)
                        kk += 1
            for (wf, wb_) in ((wq_f, wq_b), (wk_f, wk_b), (wv_f, wv_b),
                              (wo_f, wo_b)):
                nc.gpsimd.tensor_copy(wb_[:], wf[:])

            for (wb_, xb, dstT, bcol, scl) in (
                    (wq_b, xqTb, qT, bqcol, 1.0),
                    (wk_b, xkTb, kTt, bkcol, 0.125)):
                for ob in range(NRB):
                    ps = psum1.tile([P, N], F32, tag="projps")
                    for kb in range(NRB):
                        nc.tensor.matmul(ps[:],
                                         wb_[:, kb, ob * P:(ob + 1) * P],
                                         xb[:, kb, :],
                                         start=(kb == 0),
                                         stop=(kb == NRB - 1))
                    nc.scalar.activation(dstT[:, ob, :], ps[:], AF.Identity,
                                         scale=scl, bias=bcol[:, ob:ob + 1])
            for mb in range(NRB):
                ps = psum1.tile([P, D], F32, tag="projps")
                for kb in range(NRB):
                    nc.tensor.matmul(ps[:], xvTb[:, kb, mb * P:(mb + 1) * P],
                                     wv_b[:, kb, :],
                                     start=(kb == 0), stop=(kb == NRB - 1))
                nc.scalar.activation(v_sb[:, mb, :], ps[:], AF.Identity)

        # ---------------- phase 3: dw/dh banks ----------------
        bankM = persist.tile([64, N], BF16, tag="bankM")
        bankN = persist.tile([64, N], BF16, tag="bankN")
        with tc.tile_pool(name="work3", bufs=2) as work3:
            for (pcol, bank) in ((pcolm, bankM), (pcoln, bankN)):
                t_ = work3.tile([64, N], F32, tag="bk_t")
                nc.vector.tensor_scalar(t_[:32, :], l2wbc[:32, :],
                                        acol[:32, :], pcol[:32, :],
                                        ALU.mult, ALU.add)
                nc.vector.tensor_scalar(t_[32:, :], l2hbc[32:64, :],
                                        acol[32:, :], pcol[32:, :],
                                        ALU.mult, ALU.add)
                r_ = work3.tile([64, N], F32, tag="bk_r")
                nc.vector.tensor_scalar(r_[:], t_[:], MAGIC, -MAGIC,
                                        ALU.add, ALU.add)
                f_ = work3.tile([64, N], F32, tag="bk_f")
                nc.vector.tensor_tensor(f_[:], t_[:], r_[:], ALU.subtract)
                nc.scalar.activation(bank[:], f_[:], AF.Sin, scale=TWO_PI)

        # ---------------- phase 4: wg ----------------
        wgdT = persist.tile([P, H, NRB, N], BF16, tag="wgdT")
        with tc.tile_pool(name="work4", bufs=2) as work4, \
             tc.tile_pool(name="psum_u", bufs=2, space="PSUM") as psum_u, \
             tc.tile_pool(name="psum_wg", bufs=2, space="PSUM") as psum_wg:
            for rb in range(NRB):
                wgd_il = work4.tile([P, NG, N], BF16, tag="wgd_il")
                for g in range(NG):
                    off = 64 * (g // 4)
                    qq = g % 4
                    ups = psum_u.tile([P, 2, N], F32, tag="ups")
                    for ci in range(2):
                        nc.tensor.matmul(ups[:, ci, :],
                                         cR[off:off + 64, qq * P:(qq + 1) * P],
                                         dxy2[off:off + 64, rb, ci, :],
                                         start=True, stop=True)
                    rr = work4.tile([P, 2, N], F32, tag="fold_r")
                    nc.vector.tensor_scalar(rr[:], ups[:], MAGIC, -MAGIC,
                                            ALU.add, ALU.add)
                    ff = work4.tile([P, 2, N], F32, tag="fold_f")
                    nc.vector.tensor_tensor(ff[:], ups[:], rr[:],
                                            ALU.subtract)
                    habs = work4.tile([P, 2, N], F32, tag="habs")
                    nc.scalar.activation(habs[:], ff[:], AF.Abs)
                    sin4 = work4.tile([P, 4, N], BF16, tag="sin4")
                    nc.scalar.activation(sin4[:, 0:2, :], ff[:], AF.Sin,
                                         scale=TWO_PI)
                    # cos(2pi f) = sin(pi/2 - 2pi |f|)
                    nc.scalar.activation(sin4[:, 2:4, :], habs[:], AF.Sin,
                                         scale=-TWO_PI, bias=halfpi[:])
                    lhs_wh = work4.tile([64, P], BF16, tag="lhs_wh")
                    mbase = rb * P + g * GM
                    nc.gpsimd.tensor_tensor(
                        lhs_wh[:].rearrange("k (h m) -> k h m", h=H),
                        w1e_b[:].rearrange("k (h m) -> k h m", h=H),
                        bankM[:, mbase:mbase + GM][:, None, :]
                            .to_broadcast((64, H, GM)),
                        ALU.mult)
                    wgp = psum_wg.tile([P, N], F32, tag="wgp")
                    for c in range(4):
                        nc.tensor.matmul(wgp[:], wblk_b[:, c, :],
                                         sin4[:, c, :],
                                         start=(c == 0), stop=False)
                    nc.tensor.matmul(wgp[:], lhs_wh[:], bankN[:],
                                     start=False, stop=True)
                    # wgd = max(wg + bG, 1e-6) - 1 = max(wg + (bG-1), 1e-6-1)
                    nc.vector.tensor_scalar(wgd_il[:, g, :], wgp[:],
                                            bgm1[:], 1e-6 - 1.0,
                                            ALU.add, ALU.max)
                # h-major -> m-major permutation via DRAM bounce
                # (SBUF->SBUF DMA honors only one partition dim on HW, and
                # SBUF-side APs may carry only one partition dim, so the
                # write side goes per (rb, h)).
                for h in range(H):
                    nc.sync.dma_start(
                        wgd_dram[rb, h]
                            .rearrange("(g ml) n -> ml g n", g=NG),
                        wgd_il[h * GM:(h + 1) * GM, :, :])
                nc.sync.dma_start(
                    wgdT[:, :, rb, :],
                    wgd_dram[rb].rearrange("h p n -> p h n"))

        # ---------------- phase 5: attention ----------------
        with tc.tile_pool(name="work5", bufs=3) as work5, \
             tc.tile_pool(name="psum5", bufs=2, space="PSUM") as psum5, \
             tc.tile_pool(name="psum_s", bufs=1, space="PSUM") as psum_s, \
             tc.tile_pool(name="psum_av", bufs=2, space="PSUM") as psum_av, \
             tc.tile_pool(name="psum_rb", bufs=1, space="PSUM") as psum_rb, \
             tc.tile_pool(name="dbgpool", bufs=1) as dbgpool:

            objpair = persist.tile([P, NRB, N], BF16, tag="objpair")
            for rb in range(NRB):
                nc.vector.tensor_scalar(objpair[:, rb, :], objbc[:],
                                        ocol[:, rb:rb + 1], None, ALU.mult)
            # head PAIRS (2k, 2k+1) share kT/qT block ob=k at offsets 0/64
            for ob in range(H // 2):
                h0 = 2 * ob
                av = psum_av.tile([P, N], F32, tag="avps")
                sbank = psum_s.tile([H, N], F32, tag="sbank")
                for rb in range(NRB):
                    st2 = psum5.tile([P, 2, N], F32, tag="stps")
                    for hi in range(2):
                        po = hi * DK
                        nc.tensor.matmul(
                            st2[:, hi, :],
                            kTt[po:po + DK, ob, rb * P:(rb + 1) * P],
                            qT[po:po + DK, ob, :], start=True, stop=True)
                    e_ = work5.tile([P, 2, N], BF16, tag="e_t")
                    nc.scalar.activation(e_[:], st2[:], AF.Exp,
                                         bias=mcol[:, rb:rb + 1])
                    e1 = work5.tile([P, 2, N], BF16, tag="e1_t")
                    e1_eng = nc.gpsimd if (ob + rb) % 2 == 0 else nc.vector
                    e1_eng.tensor_tensor(
                        e1[:], e_[:],
                        objpair[:, rb, None, :].to_broadcast((P, 2, N)),
                        ALU.mult)
                    e2 = work5.tile([P, 2, N], BF16, tag="e2_t")
                    nc.vector.tensor_tensor(e2[:], e1[:],
                                            wgdT[:, h0:h0 + 2, rb, :],
                                            ALU.mult)
                    tt_ = work5.tile([P, 2, N], BF16, tag="tt_t")
                    nc.vector.tensor_tensor(tt_[:], e_[:], e2[:], ALU.add)
                    for hi in range(2):
                        po = hi * DK
                        nc.tensor.matmul(sbank[:],
                                         oh8_b[:, (h0 + hi) * H:
                                               (h0 + hi + 1) * H],
                                         tt_[:, hi, :],
                                         start=(rb == 0 and hi == 0),
                                         stop=(rb == NRB - 1 and hi == 1),
                                         skip_group_check=True)
                        nc.tensor.matmul(av[po:po + DK, :],
                                         v_sb[:, rb,
                                              (h0 + hi) * DK:(h0 + hi + 1) * DK],
                                         tt_[:, hi, :], start=(rb == 0),
                                         stop=(rb == NRB - 1),
                                         skip_group_check=True)
                if debug and ob == 0:
                    dbg_sb = nc.declare_dram_parameter(
                        "dbg_sbank", [H, N], F32, isOutput=True)
                    sb_c = dbgpool.tile([H, N], F32, tag="dbg_sbc")
                    nc.vector.tensor_copy(sb_c[:], sbank[:])
                    nc.sync.dma_start(dbg_sb[:], sb_c[:])
                    dbg_av = nc.declare_dram_parameter(
                        "dbg_av", [P, N], F32, isOutput=True)
                    av_c = dbgpool.tile([P, N], F32, tag="dbg_avc")
                    nc.vector.tensor_copy(av_c[:], av[:])
                    nc.sync.dma_start(dbg_av[:], av_c[:])
                sb2 = work5.tile([H, N], F32, tag="sb2")
                nc.vector.tensor_scalar(sb2[:], sbank[:],
                                        zcol[0:8, ob:ob + 1], None, ALU.add)
                rs = work5.tile([H, N], F32, tag="rs")
                nc.vector.reciprocal(rs[:], sb2[:])
                rrb = psum_rb.tile([P, N], F32, tag="rrb")
                nc.tensor.matmul(rrb[:], cF[0:8, OFF_REPL + ob * P:
                                            OFF_REPL + (ob + 1) * P],
                                 rs[0:8, :], start=True, stop=True)
                if debug and ob == 0:
                    dbg_rs = nc.declare_dram_parameter(
                        "dbg_rs", [H, N], F32, isOutput=True)
                    nc.sync.dma_start(dbg_rs[:], rs[:])
                    dbg_rrb = nc.declare_dram_parameter(
                        "dbg_rrb", [P, N], F32, isOutput=True)
                    rrb_c = dbgpool.tile([P, N], F32, tag="dbg_rrbc")
                    nc.vector.tensor_copy(rrb_c[:], rrb[:])
                    nc.sync.dma_start(dbg_rrb[:], rrb_c[:])
                rrb_sb = work5.tile([P, N], F32, tag="rrb_sb")
                nc.scalar.activation(rrb_sb[:], rrb[:], AF.Identity)
                nc.vector.tensor_tensor(ot[:, ob, :], av[:], rrb_sb[:],
                                        ALU.mult)

        if debug:
            dbg_dxy2 = nc.declare_dram_parameter(
                "dbg_dxy2", [P, NRB, 2, N], F32R, isOutput=True)
            nc.sync.dma_start(dbg_dxy2[:], dxy2[:])
            dbg_wgdT = nc.declare_dram_parameter(
                "dbg_wgdT", [P, H, NRB, N], BF16, isOutput=True)
            nc.sync.dma_start(dbg_wgdT[:], wgdT[:])
            dbg_qT = nc.declare_dram_parameter(
                "dbg_qT", [P, NRB, N], BF16, isOutput=True)
            nc.sync.dma_start(dbg_qT[:], qT[:])
            dbg_kT = nc.declare_dram_parameter(
                "dbg_kT", [P, NRB, N], BF16, isOutput=True)
            nc.sync.dma_start(dbg_kT[:], kTt[:])
            dbg_v = nc.declare_dram_parameter(
                "dbg_v", [P, NRB, D], BF16, isOutput=True)
            nc.sync.dma_start(dbg_v[:], v_sb[:])
            dbg_ot = nc.declare_dram_parameter(
                "dbg_ot", [P, NRB, N], BF16, isOutput=True)
            nc.sync.dma_start(dbg_ot[:], ot[:])

        # final projection: out[n, d]
        with tc.tile_pool(name="work6", bufs=2) as work6, \
             tc.tile_pool(name="psum6", bufs=2, space="PSUM") as psum6:
            for r in range(NRB):
                ps = psum6.tile([P, D], F32, tag="fps")
                for kt in range(NRB):
                    nc.tensor.matmul(ps[:], ot[:, kt, r * P:(r + 1) * P],
                                     wo_b[:, kt, :],
                                     start=(kt == 0), stop=(kt == NRB - 1))
                fo = work6.tile([P, D], F32, tag="fo")
                nc.vector.tensor_tensor(fo[:], ps[:], bobc, ALU.add)
                nc.sync.dma_start(out[r * P:(r + 1) * P, :], fo[:])

    if fix_waits:
        _split_multi_waits(nc)
    return nc


_NC_CACHE = {}


def kernel(**inputs):
    in_maps = _host_prep(inputs)
    if "nc" not in _NC_CACHE:
        _NC_CACHE["nc"] = build_nc()
    nc = _NC_CACHE["nc"]
    res = run_bass_kernel_spmd(nc, in_maps, list(range(B)))
    out = np.stack([res.results[b]["out"] for b in range(B)], axis=0)
    return out.astype(np.float32)


if __name__ == "__main__":
    print("kernel module ok")


# revision 28
# speedup vs baseline: 1.0280x; 1.0023x over previous
"""Trainium2 Bass kernel for BoxMultiHeadedAttention (B=8, N=512, D=512, H=8).

Sharding: data-parallel over batch — each of the 8 NeuronCores computes one
batch element end-to-end; weights replicated; no collectives.

Per-core algorithm (transposed-attention layout [m(part), n(free)]):
  * q/k/v projections on PE (bf16) from PE-transposed inputs; PSUM evictions
    on ACT (Identity with scale/bias folds the q/k biases; 1/8 folded into k).
  * geometry wg:
      - dx/dy: ln fields via ACT Square(bias=-c)/Ln + DVE sub/clamp; phase
        fractions t = (alpha_j/4pi)*dx2 by f32r selector matmuls on PE;
        magic-round fold on DVE (rr, ff) + |f|-1/4 on Pool; ONE stacked ACT
        Sin pass yields [sin(2pi f); -cos(2pi f)] with the cos sign folded
        into the WBLK weights; WG contraction on PE (bf16).
      - dw/dh: exactly separable -> rank-64 PE contraction of per-box
        sin/cos banks.
      - h-major -> m-major partition permutation via 8 merged strided DMAs
        per row-block (3-dim APs).
  * exp-domain softmax: T = E*(1 + obj*wgd); E on ACT, obj-mult on Pool,
    wgd-mult/add on DVE; row sums via PE one-hot matmul; 1/s broadcast via
    PE selector matmul (f32r); final linear on PE.
  * bv is folded into bo on the host (bo' = bo + bv @ Wo).
"""
import math
import numpy as np
from contextlib import ExitStack

import concourse.bass as bass
import concourse.mybir as mybir
import concourse.tile as tile
from concourse.bass_utils import run_bass_kernel_spmd

F32 = mybir.dt.float32
F32R = mybir.dt.float32r
BF16 = mybir.dt.bfloat16
AF = mybir.ActivationFunctionType
ALU = mybir.AluOpType

B, N, D, H = 8, 512, 512, 8
DK = D // H
P = 128
NRB = N // P
NG = 8
GM = 16
WAVE_LEN = 1000.0
MAGIC = 12582912.0
C2 = float(2.0 * math.log(0.001))
ESHIFT = -6.0
TWO_PI = float(2.0 * math.pi)

_alphas = (100.0 / (WAVE_LEN ** (np.arange(8) / 8.0))).astype(np.float64)

# const-blob column offsets (f32 blob)
OFF_IDENT = 0          # [P, 128]
OFF_OH8 = 128          # [P, 64]  col = h*8 + c
OFF_WBLK = 192         # [P, 4*128]
OFF_W1E = 704          # [64, 128]
OFF_BG = 832           # [P, 1]
OFF_ACOL = 833         # [64, 1]
OFF_PCOLM = 834        # [64, 1]
OFF_PCOLN = 835        # [64, 1]
OFF_REPL = 836         # [8, 4*128]  col = ob*128 + p; 1 iff h == ob*2+p//64
CBLOB_W = 1348
# f32r blob: selap [P, 4*128] (col q*128+c)
CBLOBR_W = 512


def _split_multi_waits(nc):
    """walrus here accepts only ONE sync-wait per ISA instruction; hoist
    extras onto NoOps inserted before the offending instruction."""
    n_fix = 0
    for blk in nc.main_func.blocks:
        insts = list(blk.instructions)
        out, dirty = [], False
        for inst in insts:
            si = inst.sync_info
            waits = list(si.on_wait) if si is not None else []
            if len(waits) > 1:
                for kk, w in enumerate(waits[:-1]):
                    out.append(mybir.InstNoOp(
                        name=f"I-waitfix-{n_fix}-{kk}", engine=inst.engine,
                        sync_info=mybir.SyncInfo(on_wait=[w], on_update=[])))
                inst.sync_info = mybir.SyncInfo(
                    on_wait=[waits[-1]], on_update=list(si.on_update))
                n_fix += 1
                dirty = True
            out.append(inst)
        if dirty:
            blk.instructions = out
    return n_fix


def _build_cblob(WG, bG):
    cb = np.zeros((P, CBLOB_W), dtype=np.float32)
    cb[:, OFF_IDENT:OFF_IDENT + P] = np.eye(P, dtype=np.float32)
    # one-hot columns for row sums: OH8[p, h*8+c] = 1 iff c == h
    for h in range(H):
        cb[:, OFF_OH8 + h * H + h] = 1.0
    # WBLK: direct sin/cos weights.  sin4 layout: [:,0,:]=sin dx,
    # [:,1,:]=sin dy, [:,2,:]=cos dx, [:,3,:]=cos dy.
    gmap = [lambda j: j, lambda j: 8 + j, lambda j: 32 + j, lambda j: 40 + j]
    gscl = [1.0, 1.0, 1.0, 1.0]
    for c in range(4):
        for ml in range(GM):
            for j in range(8):
                for h in range(H):
                    cb[ml * 8 + j, OFF_WBLK + c * P + h * GM + ml] = \
                        gscl[c] * WG[h, gmap[c](j)]
    # dw/dh rank-64 weights (angle-addition banks; unchanged from the
    # half-angle formulation since the banks encode sin/cos via phase
    # offsets in PCOL)
    w1 = np.zeros((64, H), np.float32)
    acol = np.zeros((64,), np.float32)
    pcol_m = np.zeros((64,), np.float32)
    pcol_n = np.zeros((64,), np.float32)
    for f in range(2):
        for j in range(8):
            gs = 16 + 8 * f + j
            gc = 48 + 8 * f + j
            a = _alphas[j] / (4.0 * math.pi)
            for t in range(4):
                k = (f * 8 + j) * 4 + t
                acol[k] = a
                pcol_m[k] = 0.25 if t in (0, 2) else 0.0
                if t == 0:
                    pcol_n[k] = 0.0; w1[k] = WG[:, gs]
                elif t == 1:
                    pcol_n[k] = 0.75; w1[k] = WG[:, gs]   # -cos -> +pi
                elif t == 2:
                    pcol_n[k] = 0.25; w1[k] = WG[:, gc]
                else:
                    pcol_n[k] = 0.0; w1[k] = WG[:, gc]
    cb[0:64, OFF_W1E:OFF_W1E + P] = np.repeat(w1, GM, axis=1)
    cb[:, OFF_BG] = np.repeat(bG.astype(np.float64), GM).astype(np.float32)
    cb[0:64, OFF_ACOL] = acol
    cb[0:64, OFF_PCOLM] = pcol_m
    cb[0:64, OFF_PCOLN] = pcol_n
    for ob in range(H // 2):
        for hi in range(2):
            cb[ob * 2 + hi, OFF_REPL + ob * P + hi * DK:
               OFF_REPL + ob * P + (hi + 1) * DK] = 1.0
    return cb


def _build_cblobr():
    cr = np.zeros((P, CBLOBR_W), dtype=np.float32)
    # SELAP[64*W + q*16 + ml, q*128 + ml*8 + j] = alpha_j/(4pi)
    for W in range(2):
        for q in range(4):
            for ml in range(GM):
                for j in range(8):
                    cr[64 * W + q * GM + ml, q * P + ml * 8 + j] = \
                        _alphas[j] / (4.0 * math.pi)
    return cr


def _host_prep(inputs):
    q = np.asarray(inputs["input_query"], np.float32)
    k = np.asarray(inputs["input_key"], np.float32)
    v = np.asarray(inputs["input_value"], np.float32)
    box = np.asarray(inputs["input_box"], np.float32)
    mask = np.asarray(inputs["mask"])
    nobj = np.asarray(inputs["not_objects"])
    WG = np.asarray(inputs["WG"], np.float32)
    bG = np.asarray(inputs["bG"], np.float32)
    Wo = np.asarray(inputs["Wo"], np.float32)
    bo = np.asarray(inputs["bo"], np.float32)
    bv = np.asarray(inputs["bv"], np.float32)

    x_min, y_min, x_max, y_max = [box[..., i] for i in range(4)]
    cx = (x_min + x_max) * 0.5
    cy = (y_min + y_max) * 0.5
    ww = x_max - x_min + 1.0
    hh = y_max - y_min + 1.0
    l2w = (2.0 * np.log(ww)).astype(np.float32)
    l2h = (2.0 * np.log(hh)).astype(np.float32)

    maskcol = (np.where(mask == 0, -1e9, 0.0) + ESHIFT).astype(np.float32)
    obj = (1.0 - nobj.astype(np.float32)).astype(np.float32)
    borow = (bo.astype(np.float64) + bv.astype(np.float64)
             @ Wo.astype(np.float64)).astype(np.float32)

    def col(a):  # [N] -> [P, NRB]
        return a.reshape(NRB, P).T

    shared = {
        "Wq": np.asarray(inputs["Wq"], np.float32),
        "Wk": np.asarray(inputs["Wk"], np.float32),
        "Wv": np.asarray(inputs["Wv"], np.float32),
        "Wo": Wo,
        "CBLOB": _build_cblob(WG, bG),
        "CBLOBR": _build_cblobr(),
    }
    bqc = col(np.asarray(inputs["bq"], np.float32))
    bkc = col(np.asarray(inputs["bk"], np.float32))
    in_maps = []
    for b in range(B):
        cols = np.zeros((P, 28), np.float32)
        for ob in range(4):
            cols[:, 24 + ob] = 1.0
            cols[2 * ob, 24 + ob] = 0.0
            cols[2 * ob + 1, 24 + ob] = 0.0
        cols[:, 0:4] = col(maskcol[b])
        cols[:, 4:8] = bqc
        cols[:, 8:12] = bkc
        cols[:, 12:16] = -col(cx[b])
        cols[:, 16:20] = -col(cy[b])
        cols[:, 20:24] = col(obj[b])
        rows = np.stack([cx[b], cy[b], l2w[b], l2h[b], obj[b], borow], 0)
        m = dict(shared)
        m.update({
            "xq": q[b].copy(), "xk": k[b].copy(), "xv": v[b].copy(),
            "COLS": cols, "ROWS": rows.astype(np.float32).copy(),
        })
        in_maps.append(m)
    return in_maps


def build_nc(fix_waits=True, perm_merge=True, debug=False):
    nc = bass.Bass()

    def dp(name, shape, dt=F32):
        return nc.declare_dram_parameter(name, list(shape), dt, isOutput=False)

    xq = dp("xq", (N, D)); xk = dp("xk", (N, D)); xv = dp("xv", (N, D))
    Wq = dp("Wq", (D, D)); Wk = dp("Wk", (D, D)); Wv = dp("Wv", (D, D))
    Wo = dp("Wo", (D, D))
    CBLOB = dp("CBLOB", (P, CBLOB_W))
    CBLOBR = dp("CBLOBR", (P, CBLOBR_W), F32R)
    COLS = dp("COLS", (P, 28))
    ROWS = dp("ROWS", (6, N))
    out = nc.declare_dram_parameter("out", [N, D], F32, isOutput=True)
    wgd_dram = nc.dram_tensor("wgd_scratch", [NRB, H, P, N], BF16)

    with ExitStack() as ctx:
        tc = ctx.enter_context(tile.TileContext(nc))
        const = ctx.enter_context(tc.tile_pool(name="const", bufs=1))
        persist = ctx.enter_context(tc.tile_pool(name="persist", bufs=1))

        # ---- const loads (DMA order favors phase-1/2 start) ----
        xq_sb = persist.tile([P, NRB, D], F32, tag="xq_sb")
        nc.sync.dma_start(xq_sb[:], xq.rearrange("(rb p) d -> p rb d", p=P))
        cF = const.tile([P, CBLOB_W], F32, tag="cF")
        nc.sync.dma_start(cF[:], CBLOB[:])
        cols_t = const.tile([P, 28], F32, tag="cols")
        nc.sync.dma_start(cols_t[:], COLS[:])
        rows_t = const.tile([P, 6, N], F32, tag="rows")
        nc.sync.dma_start(rows_t[:], ROWS[None, :, :].to_broadcast((P, 6, N)))
        wq_f = persist.tile([P, NRB, D], F32, tag="wq_f")
        nc.sync.dma_start(wq_f[:], Wq.rearrange("(kb p) d -> p kb d", p=P))
        xk_sb = persist.tile([P, NRB, D], F32, tag="xk_sb")
        nc.sync.dma_start(xk_sb[:], xk.rearrange("(rb p) d -> p rb d", p=P))
        xv_sb = persist.tile([P, NRB, D], F32, tag="xv_sb")
        nc.sync.dma_start(xv_sb[:], xv.rearrange("(rb p) d -> p rb d", p=P))
        wk_f = persist.tile([P, NRB, D], F32, tag="wk_f")
        nc.sync.dma_start(wk_f[:], Wk.rearrange("(kb p) d -> p kb d", p=P))
        wv_f = persist.tile([P, NRB, D], F32, tag="wv_f")
        nc.sync.dma_start(wv_f[:], Wv.rearrange("(kb p) d -> p kb d", p=P))
        cR = const.tile([P, CBLOBR_W], F32R, tag="cR")
        nc.sync.dma_start(cR[:], CBLOBR[:])
        wo_f = persist.tile([P, NRB, D], F32, tag="wo_f")
        nc.sync.dma_start(wo_f[:], Wo.rearrange("(kb p) d -> p kb d", p=P))

        ident = cF[:, OFF_IDENT:OFF_IDENT + P]
        mcol = cols_t[:, 0:4]
        bqcol = cols_t[:, 4:8]
        bkcol = cols_t[:, 8:12]
        negcx = cols_t[:, 12:16]
        negcy = cols_t[:, 16:20]
        ocol = cols_t[:, 20:24]
        zcol = cols_t[:, 24:28]
        cxbc = rows_t[:, 0, :]
        cybc = rows_t[:, 1, :]
        l2wbc = rows_t[:, 2, :]
        l2hbc = rows_t[:, 3, :]
        objbc_f = rows_t[:, 4, :]
        bobc = rows_t[:, 5, :]

        # small const casts / derived
        oh8_b = const.tile([P, H * H], BF16, tag="oh8b")
        nc.vector.tensor_copy(oh8_b[:], cF[:, OFF_OH8:OFF_OH8 + H * H])
        wblk_b = const.tile([P, 4, P], BF16, tag="wblkb")
        for c in range(4):
            nc.gpsimd.tensor_copy(wblk_b[:, c, :],
                                  cF[:, OFF_WBLK + c * P:OFF_WBLK + (c + 1) * P])
        w1e_b = const.tile([64, P], BF16, tag="w1eb")
        nc.gpsimd.tensor_copy(w1e_b[:], cF[0:64, OFF_W1E:OFF_W1E + P])
        objbc = const.tile([P, N], BF16, tag="objbc")
        nc.gpsimd.tensor_copy(objbc[:], objbc_f[:])
        halfpi = const.tile([P, 1], F32, tag="halfpi")
        nc.vector.memset(halfpi[:], float(math.pi / 2.0))
        bgm1 = const.tile([P, 1], F32, tag="bgm1")
        nc.vector.tensor_scalar(bgm1[:], cF[:, OFF_BG:OFF_BG + 1], -1.0, None,
                                ALU.add)
        acol = cF[0:64, OFF_ACOL:OFF_ACOL + 1]
        pcolm = cF[0:64, OFF_PCOLM:OFF_PCOLM + 1]
        pcoln = cF[0:64, OFF_PCOLN:OFF_PCOLN + 1]

        # ---------------- phases 1+2 (shared scope so they overlap) -------
        dxy2 = persist.tile([P, NRB, 2, N], F32R, tag="dxy2")
        bankM = persist.tile([64, N], BF16, tag="bankM")
        bankN = persist.tile([64, N], BF16, tag="bankN")
        qT = persist.tile([P, NRB, N], BF16, tag="qT")
        kTt = persist.tile([P, NRB, N], BF16, tag="kT")
        v_sb = persist.tile([P, NRB, D], BF16, tag="v_sb")
        wo_b = persist.tile([P, NRB, D], BF16, tag="wob")
        ot = persist.tile([P, NRB, N], BF16, tag="ot")

        with tc.tile_pool(name="tpool", bufs=1) as tpool, \
             tc.tile_pool(name="work2", bufs=3) as work2, \
             tc.tile_pool(name="work3", bufs=1) as work3, \
             tc.tile_pool(name="work1", bufs=2) as work1, \
             tc.tile_pool(name="psum1", bufs=3, space="PSUM") as psum1:
            # phase 2: ln fields (ACT Square/Ln + DVE sub/clamp)
            for rb in range(NRB):
                for (ci, cbc, ncol, l2bc) in ((0, cxbc, negcx, l2wbc),
                                              (1, cybc, negcy, l2hbc)):
                    d2 = work2.tile([P, N], F32, tag="geo_d2")
                    nc.scalar.activation(d2[:], cbc, AF.Square,
                                         bias=ncol[:, rb:rb + 1])
                    l2t = work2.tile([P, N], F32, tag="geo_l2")
                    nc.scalar.activation(l2t[:], d2[:], AF.Ln)
                    g_ = work2.tile([P, N], F32, tag="geo_g")
                    nc.vector.tensor_tensor(g_[:], l2t[:], l2bc, ALU.subtract)
                    nc.vector.tensor_scalar_max(dxy2[:, rb, ci, :], g_[:], C2)

            # phase 3: dw/dh banks (early; DVE idle at start)
            for (pcol, bank) in ((pcolm, bankM), (pcoln, bankN)):
                t_ = work3.tile([64, N], F32, tag="bk_t")
                nc.vector.tensor_scalar(t_[:32, :], l2wbc[:32, :],
                                        acol[:32, :], pcol[:32, :],
                                        ALU.mult, ALU.add)
                nc.vector.tensor_scalar(t_[32:, :], l2hbc[32:64, :],
                                        acol[32:, :], pcol[32:, :],
                                        ALU.mult, ALU.add)
                r_ = work3.tile([64, N], F32, tag="bk_r")
                nc.vector.tensor_scalar(r_[:], t_[:], MAGIC, -MAGIC,
                                        ALU.add, ALU.add)
                f_ = work3.tile([64, N], F32, tag="bk_f")
                nc.vector.tensor_tensor(f_[:], t_[:], r_[:], ALU.subtract)
                nc.scalar.activation(bank[:], f_[:], AF.Sin, scale=TWO_PI)

            # phase 1: transposes + projections
            xqTb = tpool.tile([P, NRB, N], BF16, tag="xqTb")
            xkTb = tpool.tile([P, NRB, N], BF16, tag="xkTb")
            xvTb = tpool.tile([P, NRB, N], BF16, tag="xvTb")
            wq_b = tpool.tile([P, NRB, D], BF16, tag="wqb")
            wk_b = tpool.tile([P, NRB, D], BF16, tag="wkb")
            wv_b = tpool.tile([P, NRB, D], BF16, tag="wvb")

            kk = 0
            for (xs, dstb) in ((xq_sb, xqTb), (xk_sb, xkTb), (xv_sb, xvTb)):
                for rb in range(NRB):
                    for cb in range(NRB):
                        tp = psum1.tile([P, P], F32, tag="tp")
                        nc.tensor.transpose(tp[:], xs[:, rb, cb * P:(cb + 1) * P],
                                            ident)
                        dst = dstb[:, cb, rb * P:(rb + 1) * P]
                        if kk % 2 == 0:
                            nc.vector.tensor_copy(dst, tp[:])
                        else:
                            nc.scalar.activation(dst, tp[:], AF.Identity)
                        kk += 1
            for (wf, wb_) in ((wq_f, wq_b), (wk_f, wk_b), (wv_f, wv_b),
                              (wo_f, wo_b)):
                nc.gpsimd.tensor_copy(wb_[:], wf[:])

            for (wb_, xb, dstT, bcol, scl) in (
                    (wq_b, xqTb, qT, bqcol, 1.0),
                    (wk_b, xkTb, kTt, bkcol, 0.125)):
                for ob in range(NRB):
                    ps = psum1.tile([P, N], F32, tag="projps")
                    for kb in range(NRB):
                        nc.tensor.matmul(ps[:],
                                         wb_[:, kb, ob * P:(ob + 1) * P],
                                         xb[:, kb, :],
                                         start=(kb == 0),
                                         stop=(kb == NRB - 1))
                    nc.scalar.activation(dstT[:, ob, :], ps[:], AF.Identity,
                                         scale=scl, bias=bcol[:, ob:ob + 1])
            for mb in range(NRB):
                ps = psum1.tile([P, D], F32, tag="projps")
                for kb in range(NRB):
                    nc.tensor.matmul(ps[:], xvTb[:, kb, mb * P:(mb + 1) * P],
                                     wv_b[:, kb, :],
                                     start=(kb == 0), stop=(kb == NRB - 1))
                nc.scalar.activation(v_sb[:, mb, :], ps[:], AF.Identity)

        # ---------------- phase 4: wg ----------------
        wgdT = persist.tile([P, H, NRB, N], BF16, tag="wgdT")
        with tc.tile_pool(name="work4", bufs=2) as work4, \
             tc.tile_pool(name="psum_u", bufs=2, space="PSUM") as psum_u, \
             tc.tile_pool(name="psum_wg", bufs=2, space="PSUM") as psum_wg:
            for rb in range(NRB):
                wgd_il = work4.tile([P, NG, N], BF16, tag="wgd_il")
                for g in range(NG):
                    off = 64 * (g // 4)
                    qq = g % 4
                    ups = psum_u.tile([P, 2, N], F32, tag="ups")
                    for ci in range(2):
                        nc.tensor.matmul(ups[:, ci, :],
                                         cR[off:off + 64, qq * P:(qq + 1) * P],
                                         dxy2[off:off + 64, rb, ci, :],
                                         start=True, stop=True)
                    rr = work4.tile([P, 2, N], F32, tag="fold_r")
                    nc.vector.tensor_scalar(rr[:], ups[:], MAGIC, -MAGIC,
                                            ALU.add, ALU.add)
                    ff = work4.tile([P, 2, N], F32, tag="fold_f")
                    nc.vector.tensor_tensor(ff[:], ups[:], rr[:],
                                            ALU.subtract)
                    habs = work4.tile([P, 2, N], F32, tag="habs")
                    nc.scalar.activation(habs[:], ff[:], AF.Abs)
                    sin4 = work4.tile([P, 4, N], BF16, tag="sin4")
                    nc.scalar.activation(sin4[:, 0:2, :], ff[:], AF.Sin,
                                         scale=TWO_PI)
                    # cos(2pi f) = sin(pi/2 - 2pi |f|)
                    nc.scalar.activation(sin4[:, 2:4, :], habs[:], AF.Sin,
                                         scale=-TWO_PI, bias=halfpi[:])
                    lhs_wh = work4.tile([64, P], BF16, tag="lhs_wh")
                    mbase = rb * P + g * GM
                    nc.gpsimd.tensor_tensor(
                        lhs_wh[:].rearrange("k (h m) -> k h m", h=H),
                        w1e_b[:].rearrange("k (h m) -> k h m", h=H),
                        bankM[:, mbase:mbase + GM][:, None, :]
                            .to_broadcast((64, H, GM)),
                        ALU.mult)
                    wgp = psum_wg.tile([P, N], F32, tag="wgp")
                    for c in range(4):
                        nc.tensor.matmul(wgp[:], wblk_b[:, c, :],
                                         sin4[:, c, :],
                                         start=(c == 0), stop=False)
                    nc.tensor.matmul(wgp[:], lhs_wh[:], bankN[:],
                                     start=False, stop=True)
                    # wgd = max(wg + bG, 1e-6) - 1 = max(wg + (bG-1), 1e-6-1)
                    nc.vector.tensor_scalar(wgd_il[:, g, :], wgp[:],
                                            bgm1[:], 1e-6 - 1.0,
                                            ALU.add, ALU.max)
                # h-major -> m-major permutation via DRAM bounce
                # (SBUF->SBUF DMA honors only one partition dim on HW, and
                # SBUF-side APs may carry only one partition dim, so the
                # write side goes per (rb, h)).
                for h in range(H):
                    nc.sync.dma_start(
                        wgd_dram[rb, h]
                            .rearrange("(g ml) n -> ml g n", g=NG),
                        wgd_il[h * GM:(h + 1) * GM, :, :])
                nc.sync.dma_start(
                    wgdT[:, :, rb, :],
                    wgd_dram[rb].rearrange("h p n -> p h n"))

        # ---------------- phase 5: attention ----------------
        with tc.tile_pool(name="work5", bufs=3) as work5, \
             tc.tile_pool(name="psum5", bufs=2, space="PSUM") as psum5, \
             tc.tile_pool(name="psum_s", bufs=1, space="PSUM") as psum_s, \
             tc.tile_pool(name="psum_av", bufs=2, space="PSUM") as psum_av, \
             tc.tile_pool(name="psum_rb", bufs=1, space="PSUM") as psum_rb, \
             tc.tile_pool(name="dbgpool", bufs=1) as dbgpool:

            objpair = persist.tile([P, NRB, N], BF16, tag="objpair")
            for rb in range(NRB):
                nc.vector.tensor_scalar(objpair[:, rb, :], objbc[:],
                                        ocol[:, rb:rb + 1], None, ALU.mult)
            # head PAIRS (2k, 2k+1) share kT/qT block ob=k at offsets 0/64
            for ob in range(H // 2):
                h0 = 2 * ob
                av = psum_av.tile([P, N], F32, tag="avps")
                sbank = psum_s.tile([H, N], F32, tag="sbank")
                for rb in range(NRB):
                    st2 = psum5.tile([P, 2, N], F32, tag="stps")
                    for hi in range(2):
                        po = hi * DK
                        nc.tensor.matmul(
                            st2[:, hi, :],
                            kTt[po:po + DK, ob, rb * P:(rb + 1) * P],
                            qT[po:po + DK, ob, :], start=True, stop=True)
                    e_ = work5.tile([P, 2, N], BF16, tag="e_t")
                    nc.scalar.activation(e_[:], st2[:], AF.Exp,
                                         bias=mcol[:, rb:rb + 1])
                    e1 = work5.tile([P, 2, N], BF16, tag="e1_t")
                    e1_eng = nc.gpsimd if (ob + rb) % 2 == 0 else nc.vector
                    e1_eng.tensor_tensor(
                        e1[:], e_[:],
                        objpair[:, rb, None, :].to_broadcast((P, 2, N)),
                        ALU.mult)
                    e2 = work5.tile([P, 2, N], BF16, tag="e2_t")
                    nc.vector.tensor_tensor(e2[:], e1[:],
                                            wgdT[:, h0:h0 + 2, rb, :],
                                            ALU.mult)
                    tt_ = work5.tile([P, 2, N], BF16, tag="tt_t")
                    nc.vector.tensor_tensor(tt_[:], e_[:], e2[:], ALU.add)
                    for hi in range(2):
                        po = hi * DK
                        nc.tensor.matmul(sbank[:],
                                         oh8_b[:, (h0 + hi) * H:
                                               (h0 + hi + 1) * H],
                                         tt_[:, hi, :],
                                         start=(rb == 0 and hi == 0),
                                         stop=(rb == NRB - 1 and hi == 1),
                                         skip_group_check=True)
                        nc.tensor.matmul(av[po:po + DK, :],
                                         v_sb[:, rb,
                                              (h0 + hi) * DK:(h0 + hi + 1) * DK],
                                         tt_[:, hi, :], start=(rb == 0),
                                         stop=(rb == NRB - 1),
                                         skip_group_check=True)
                if debug and ob == 0:
                    dbg_sb = nc.declare_dram_parameter(
                        "dbg_sbank", [H, N], F32, isOutput=True)
                    sb_c = dbgpool.tile([H, N], F32, tag="dbg_sbc")
                    nc.vector.tensor_copy(sb_c[:], sbank[:])
                    nc.sync.dma_start(dbg_sb[:], sb_c[:])
                    dbg_av = nc.declare_dram_parameter(
                        "dbg_av", [P, N], F32, isOutput=True)
                    av_c = dbgpool.tile([P, N], F32, tag="dbg_avc")
                    nc.vector.tensor_copy(av_c[:], av[:])
                    nc.sync.dma_start(dbg_av[:], av_c[:])
                sb2 = work5.tile([H, N], F32, tag="sb2")
                nc.vector.tensor_scalar(sb2[:], sbank[:],
                                        zcol[0:8, ob:ob + 1], None, ALU.add)
                rs = work5.tile([H, N], F32, tag="rs")
                nc.vector.reciprocal(rs[:], sb2[:])
                rrb = psum_rb.tile([P, N], F32, tag="rrb")
                nc.tensor.matmul(rrb[:], cF[0:8, OFF_REPL + ob * P:
                                            OFF_REPL + (ob + 1) * P],
                                 rs[0:8, :], start=True, stop=True)
                if debug and ob == 0:
                    dbg_rs = nc.declare_dram_parameter(
                        "dbg_rs", [H, N], F32, isOutput=True)
                    nc.sync.dma_start(dbg_rs[:], rs[:])
                    dbg_rrb = nc.declare_dram_parameter(
                        "dbg_rrb", [P, N], F32, isOutput=True)
                    rrb_c = dbgpool.tile([P, N], F32, tag="dbg_rrbc")
                    nc.vector.tensor_copy(rrb_c[:], rrb[:])
                    nc.sync.dma_start(dbg_rrb[:], rrb_c[:])
                rrb_sb = work5.tile([P, N], F32, tag="rrb_sb")
                nc.scalar.activation(rrb_sb[:], rrb[:], AF.Identity)
                nc.vector.tensor_tensor(ot[:, ob, :], av[:], rrb_sb[:],
                                        ALU.mult)

        if debug:
            dbg_dxy2 = nc.declare_dram_parameter(
                "dbg_dxy2", [P, NRB, 2, N], F32R, isOutput=True)
            nc.sync.dma_start(dbg_dxy2[:], dxy2[:])
            dbg_wgdT = nc.declare_dram_parameter(
                "dbg_wgdT", [P, H, NRB, N], BF16, isOutput=True)
            nc.sync.dma_start(dbg_wgdT[:], wgdT[:])
            dbg_qT = nc.declare_dram_parameter(
                "dbg_qT", [P, NRB, N], BF16, isOutput=True)
            nc.sync.dma_start(dbg_qT[:], qT[:])
            dbg_kT = nc.declare_dram_parameter(
                "dbg_kT", [P, NRB, N], BF16, isOutput=True)
            nc.sync.dma_start(dbg_kT[:], kTt[:])
            dbg_v = nc.declare_dram_parameter(
                "dbg_v", [P, NRB, D], BF16, isOutput=True)
            nc.sync.dma_start(dbg_v[:], v_sb[:])
            dbg_ot = nc.declare_dram_parameter(
                "dbg_ot", [P, NRB, N], BF16, isOutput=True)
            nc.sync.dma_start(dbg_ot[:], ot[:])

        # final projection: out[n, d]
        with tc.tile_pool(name="work6", bufs=2) as work6, \
             tc.tile_pool(name="psum6", bufs=2, space="PSUM") as psum6:
            for r in range(NRB):
                ps = psum6.tile([P, D], F32, tag="fps")
                for kt in range(NRB):
                    nc.tensor.matmul(ps[:], ot[:, kt, r * P:(r + 1) * P],
                                     wo_b[:, kt, :],
                                     start=(kt == 0), stop=(kt == NRB - 1))
                fo = work6.tile([P, D], F32, tag="fo")
                nc.vector.tensor_tensor(fo[:], ps[:], bobc, ALU.add)
                nc.sync.dma_start(out[r * P:(r + 1) * P, :], fo[:])

    if fix_waits:
        _split_multi_waits(nc)
    return nc


_NC_CACHE = {}


def kernel(**inputs):
    in_maps = _host_prep(inputs)
    if "nc" not in _NC_CACHE:
        _NC_CACHE["nc"] = build_nc()
    nc = _NC_CACHE["nc"]
    res = run_bass_kernel_spmd(nc, in_maps, list(range(B)))
    out = np.stack([res.results[b]["out"] for b in range(B)], axis=0)
    return out.astype(np.float32)


if __name__ == "__main__":
    print("kernel module ok")


# revision 37
# speedup vs baseline: 1.0672x; 1.0381x over previous
"""Trainium2 Bass kernel for BoxMultiHeadedAttention (B=8, N=512, D=512, H=8).

Sharding: data-parallel over batch — each of the 8 NeuronCores computes one
batch element end-to-end; weights replicated; no collectives.

Per-core algorithm (transposed-attention layout [m(part), n(free)]):
  * q/k/v projections on PE (bf16) from PE-transposed inputs; PSUM evictions
    on ACT (Identity with scale/bias folds the q/k biases; 1/8 folded into k).
  * geometry wg:
      - dx/dy: ln fields via ACT Square(bias=-c)/Ln + DVE sub/clamp; phase
        fractions t = (alpha_j/4pi)*dx2 by f32r selector matmuls on PE;
        magic-round fold on DVE (rr, ff) + |f|-1/4 on Pool; ONE stacked ACT
        Sin pass yields [sin(2pi f); -cos(2pi f)] with the cos sign folded
        into the WBLK weights; WG contraction on PE (bf16).
      - dw/dh: exactly separable -> rank-64 PE contraction of per-box
        sin/cos banks.
      - h-major -> m-major partition permutation via 8 merged strided DMAs
        per row-block (3-dim APs).
  * exp-domain softmax: T = E*(1 + obj*wgd); E on ACT, obj-mult on Pool,
    wgd-mult/add on DVE; row sums via PE one-hot matmul; 1/s broadcast via
    PE selector matmul (f32r); final linear on PE.
  * bv is folded into bo on the host (bo' = bo + bv @ Wo).
"""
import math
import numpy as np
from contextlib import ExitStack

import concourse.bass as bass
import concourse.mybir as mybir
import concourse.tile as tile
from concourse.bass_utils import run_bass_kernel_spmd

F32 = mybir.dt.float32
F32R = mybir.dt.float32r
BF16 = mybir.dt.bfloat16
AF = mybir.ActivationFunctionType
ALU = mybir.AluOpType

B, N, D, H = 8, 512, 512, 8
DK = D // H
P = 128
NRB = N // P
NG = 8
GM = 16
WAVE_LEN = 1000.0
MAGIC = 12582912.0
C2 = float(2.0 * math.log(0.001))
ESHIFT = -6.0
TWO_PI = float(2.0 * math.pi)

_alphas = (100.0 / (WAVE_LEN ** (np.arange(8) / 8.0))).astype(np.float64)

# const-blob column offsets (f32 blob)
OFF_IDENT = 0          # [P, 128]
OFF_OH8 = 128          # [P, 64]  col = h*8 + c
OFF_WBLK = 192         # [P, 4*128]
OFF_W1E = 704          # [64, 128]
OFF_BG = 832           # [P, 1]
OFF_ACOL = 833         # [64, 1]
OFF_PCOLM = 834        # [64, 1]
OFF_PCOLN = 835        # [64, 1]
OFF_REPL = 836         # [8, 4*128]  col = ob*128 + p; 1 iff h == ob*2+p//64
CBLOB_W = 1348
# f32r blob: selap [P, 4*128] (col q*128+c)
CBLOBR_W = 512


def _split_multi_waits(nc):
    """walrus here accepts only ONE sync-wait per ISA instruction; hoist
    extras onto NoOps inserted before the offending instruction."""
    n_fix = 0
    for blk in nc.main_func.blocks:
        insts = list(blk.instructions)
        out, dirty = [], False
        for inst in insts:
            si = inst.sync_info
            waits = list(si.on_wait) if si is not None else []
            if len(waits) > 1:
                for kk, w in enumerate(waits[:-1]):
                    out.append(mybir.InstNoOp(
                        name=f"I-waitfix-{n_fix}-{kk}", engine=inst.engine,
                        sync_info=mybir.SyncInfo(on_wait=[w], on_update=[])))
                inst.sync_info = mybir.SyncInfo(
                    on_wait=[waits[-1]], on_update=list(si.on_update))
                n_fix += 1
                dirty = True
            out.append(inst)
        if dirty:
            blk.instructions = out
    return n_fix


def _build_cblob(WG, bG):
    cb = np.zeros((P, CBLOB_W), dtype=np.float32)
    cb[:, OFF_IDENT:OFF_IDENT + P] = np.eye(P, dtype=np.float32)
    # one-hot columns for row sums: OH8[p, h*8+c] = 1 iff c == h
    for h in range(H):
        cb[:, OFF_OH8 + h * H + h] = 1.0
    # WBLK: direct sin/cos weights.  sin4 layout: [:,0,:]=sin dx,
    # [:,1,:]=sin dy, [:,2,:]=cos dx, [:,3,:]=cos dy.
    gmap = [lambda j: j, lambda j: 8 + j, lambda j: 32 + j, lambda j: 40 + j]
    gscl = [1.0, 1.0, 1.0, 1.0]
    for c in range(4):
        for ml in range(GM):
            for j in range(8):
                for h in range(H):
                    cb[ml * 8 + j, OFF_WBLK + c * P + h * GM + ml] = \
                        gscl[c] * WG[h, gmap[c](j)]
    # dw/dh rank-64 weights (angle-addition banks; unchanged from the
    # half-angle formulation since the banks encode sin/cos via phase
    # offsets in PCOL)
    w1 = np.zeros((64, H), np.float32)
    acol = np.zeros((64,), np.float32)
    pcol_m = np.zeros((64,), np.float32)
    pcol_n = np.zeros((64,), np.float32)
    for f in range(2):
        for j in range(8):
            gs = 16 + 8 * f + j
            gc = 48 + 8 * f + j
            a = _alphas[j] / (4.0 * math.pi)
            for t in range(4):
                k = (f * 8 + j) * 4 + t
                acol[k] = a
                pcol_m[k] = 0.25 if t in (0, 2) else 0.0
                if t == 0:
                    pcol_n[k] = 0.0; w1[k] = WG[:, gs]
                elif t == 1:
                    pcol_n[k] = 0.75; w1[k] = WG[:, gs]   # -cos -> +pi
                elif t == 2:
                    pcol_n[k] = 0.25; w1[k] = WG[:, gc]
                else:
                    pcol_n[k] = 0.0; w1[k] = WG[:, gc]
    cb[0:64, OFF_W1E:OFF_W1E + P] = np.repeat(w1, GM, axis=1)
    cb[:, OFF_BG] = np.repeat(bG.astype(np.float64), GM).astype(np.float32)
    cb[0:64, OFF_ACOL] = acol
    cb[0:64, OFF_PCOLM] = pcol_m
    cb[0:64, OFF_PCOLN] = pcol_n
    for ob in range(H // 2):
        for hi in range(2):
            cb[ob * 2 + hi, OFF_REPL + ob * P + hi * DK:
               OFF_REPL + ob * P + (hi + 1) * DK] = 1.0
    return cb


def _build_cblobr():
    cr = np.zeros((P, CBLOBR_W), dtype=np.float32)
    # SELAP[64*W + q*16 + ml, q*128 + ml*8 + j] = alpha_j/(4pi)
    for W in range(2):
        for q in range(4):
            for ml in range(GM):
                for j in range(8):
                    cr[64 * W + q * GM + ml, q * P + ml * 8 + j] = \
                        _alphas[j] / (4.0 * math.pi)
    return cr


def _host_prep(inputs):
    q = np.asarray(inputs["input_query"], np.float32)
    k = np.asarray(inputs["input_key"], np.float32)
    v = np.asarray(inputs["input_value"], np.float32)
    box = np.asarray(inputs["input_box"], np.float32)
    mask = np.asarray(inputs["mask"])
    nobj = np.asarray(inputs["not_objects"])
    WG = np.asarray(inputs["WG"], np.float32)
    bG = np.asarray(inputs["bG"], np.float32)
    Wo = np.asarray(inputs["Wo"], np.float32)
    bo = np.asarray(inputs["bo"], np.float32)
    bv = np.asarray(inputs["bv"], np.float32)

    x_min, y_min, x_max, y_max = [box[..., i] for i in range(4)]
    cx = (x_min + x_max) * 0.5
    cy = (y_min + y_max) * 0.5
    ww = x_max - x_min + 1.0
    hh = y_max - y_min + 1.0
    l2w = (2.0 * np.log(ww)).astype(np.float32)
    l2h = (2.0 * np.log(hh)).astype(np.float32)

    maskcol = (np.where(mask == 0, -1e9, 0.0) + ESHIFT).astype(np.float32)
    obj = (1.0 - nobj.astype(np.float32)).astype(np.float32)
    borow = (bo.astype(np.float64) + bv.astype(np.float64)
             @ Wo.astype(np.float64)).astype(np.float32)

    def col(a):  # [N] -> [P, NRB]
        return a.reshape(NRB, P).T

    shared = {
        "Wq": np.asarray(inputs["Wq"], np.float32),
        "Wk": np.asarray(inputs["Wk"], np.float32),
        "Wv": np.asarray(inputs["Wv"], np.float32),
        "Wo": Wo,
        "CBLOB": _build_cblob(WG, bG),
        "CBLOBR": _build_cblobr(),
    }
    bqc = col(np.asarray(inputs["bq"], np.float32))
    bkc = col(np.asarray(inputs["bk"], np.float32))
    in_maps = []
    for b in range(B):
        cols = np.zeros((P, 28), np.float32)
        for ob in range(4):
            cols[:, 24 + ob] = 1.0
            cols[2 * ob, 24 + ob] = 0.0
            cols[2 * ob + 1, 24 + ob] = 0.0
        cols[:, 0:4] = col(maskcol[b])
        cols[:, 4:8] = bqc
        cols[:, 8:12] = bkc
        cols[:, 12:16] = -col(cx[b])
        cols[:, 16:20] = -col(cy[b])
        cols[:, 20:24] = col(obj[b])
        rows = np.stack([cx[b], cy[b], l2w[b], l2h[b], obj[b], borow], 0)
        m = dict(shared)
        m.update({
            "xq": q[b].copy(), "xk": k[b].copy(), "xv": v[b].copy(),
            "COLS": cols, "ROWS": rows.astype(np.float32).copy(),
        })
        in_maps.append(m)
    return in_maps


def build_nc(fix_waits=True, perm_merge=True, debug=False):
    nc = bass.Bass()

    def dp(name, shape, dt=F32):
        return nc.declare_dram_parameter(name, list(shape), dt, isOutput=False)

    xq = dp("xq", (N, D)); xk = dp("xk", (N, D)); xv = dp("xv", (N, D))
    Wq = dp("Wq", (D, D)); Wk = dp("Wk", (D, D)); Wv = dp("Wv", (D, D))
    Wo = dp("Wo", (D, D))
    CBLOB = dp("CBLOB", (P, CBLOB_W))
    CBLOBR = dp("CBLOBR", (P, CBLOBR_W), F32R)
    COLS = dp("COLS", (P, 28))
    ROWS = dp("ROWS", (6, N))
    out = nc.declare_dram_parameter("out", [N, D], F32, isOutput=True)
    wgd_dram = nc.dram_tensor("wgd_scratch", [NRB, H, P, N], BF16)

    with ExitStack() as ctx:
        tc = ctx.enter_context(tile.TileContext(nc))
        const = ctx.enter_context(tc.tile_pool(name="const", bufs=1))
        persist = ctx.enter_context(tc.tile_pool(name="persist", bufs=1))

        # ---- const loads (DMA order favors phase-1/2 start) ----
        loadp = ctx.enter_context(tc.tile_pool(name="loadp", bufs=1))
        xq_sb = loadp.tile([P, NRB, D], F32, tag="xq_sb")
        nc.sync.dma_start(xq_sb[:], xq.rearrange("(rb p) d -> p rb d", p=P))
        cF = const.tile([P, CBLOB_W], F32, tag="cF")
        # ident first: unblocks the PE transposes ~4us earlier than the
        # full blob would
        nc.sync.dma_start(cF[:, 0:P], CBLOB[:, 0:P])
        cols_t = const.tile([P, 28], F32, tag="cols")
        nc.sync.dma_start(cols_t[:], COLS[:])
        rows_t = const.tile([P, 6, N], F32, tag="rows")
        nc.sync.dma_start(rows_t[:, 0:4, :],
                          ROWS[None, 0:4, :].to_broadcast((P, 4, N)))
        nc.sync.dma_start(cF[:, P:], CBLOB[:, P:])
        nc.sync.dma_start(rows_t[:, 4:6, :],
                          ROWS[None, 4:6, :].to_broadcast((P, 2, N)))
        wq_f = loadp.tile([P, NRB, D], F32, tag="wq_f")
        nc.sync.dma_start(wq_f[:], Wq.rearrange("(kb p) d -> p kb d", p=P))
        xk_sb = loadp.tile([P, NRB, D], F32, tag="xk_sb")
        nc.sync.dma_start(xk_sb[:], xk.rearrange("(rb p) d -> p rb d", p=P))
        xv_sb = loadp.tile([P, NRB, D], F32, tag="xv_sb")
        nc.sync.dma_start(xv_sb[:], xv.rearrange("(rb p) d -> p rb d", p=P))
        wk_f = loadp.tile([P, NRB, D], F32, tag="wk_f")
        nc.sync.dma_start(wk_f[:], Wk.rearrange("(kb p) d -> p kb d", p=P))
        wv_f = loadp.tile([P, NRB, D], F32, tag="wv_f")
        nc.sync.dma_start(wv_f[:], Wv.rearrange("(kb p) d -> p kb d", p=P))
        cR = const.tile([P, CBLOBR_W], F32R, tag="cR")
        nc.sync.dma_start(cR[:], CBLOBR[:])
        wo_f = loadp.tile([P, NRB, D], F32, tag="wo_f")
        nc.sync.dma_start(wo_f[:], Wo.rearrange("(kb p) d -> p kb d", p=P))

        ident = cF[:, OFF_IDENT:OFF_IDENT + P]
        mcol = cols_t[:, 0:4]
        bqcol = cols_t[:, 4:8]
        bkcol = cols_t[:, 8:12]
        negcx = cols_t[:, 12:16]
        negcy = cols_t[:, 16:20]
        ocol = cols_t[:, 20:24]
        zcol = cols_t[:, 24:28]
        cxbc = rows_t[:, 0, :]
        cybc = rows_t[:, 1, :]
        l2wbc = rows_t[:, 2, :]
        l2hbc = rows_t[:, 3, :]
        objbc_f = rows_t[:, 4, :]
        bobc = rows_t[:, 5, :]

        # small const casts / derived
        oh8_b = const.tile([P, H * H], BF16, tag="oh8b")
        nc.vector.tensor_copy(oh8_b[:], cF[:, OFF_OH8:OFF_OH8 + H * H])
        wblk_b = const.tile([P, 4, P], BF16, tag="wblkb")
        for c in range(4):
            nc.gpsimd.tensor_copy(wblk_b[:, c, :],
                                  cF[:, OFF_WBLK + c * P:OFF_WBLK + (c + 1) * P])
        w1e_b = const.tile([64, P], BF16, tag="w1eb")
        nc.gpsimd.tensor_copy(w1e_b[:], cF[0:64, OFF_W1E:OFF_W1E + P])
        objbc = const.tile([P, N], BF16, tag="objbc")
        nc.gpsimd.tensor_copy(objbc[:], objbc_f[:])
        halfpi = const.tile([P, 1], F32, tag="halfpi")
        nc.vector.memset(halfpi[:], float(math.pi / 2.0))
        bgm1 = const.tile([P, 1], F32, tag="bgm1")
        nc.vector.tensor_scalar(bgm1[:], cF[:, OFF_BG:OFF_BG + 1], -1.0, None,
                                ALU.add)
        acol = cF[0:64, OFF_ACOL:OFF_ACOL + 1]
        pcolm = cF[0:64, OFF_PCOLM:OFF_PCOLM + 1]
        pcoln = cF[0:64, OFF_PCOLN:OFF_PCOLN + 1]

        # ---------------- phases 1+2 (shared scope so they overlap) -------
        dxy2 = persist.tile([P, NRB, 2, N], F32R, tag="dxy2")
        bankM = persist.tile([64, N], BF16, tag="bankM")
        bankN = persist.tile([64, N], BF16, tag="bankN")
        qT = persist.tile([P, NRB, N], BF16, tag="qT")
        kTt = persist.tile([P, NRB, N], BF16, tag="kT")
        v_sb = persist.tile([P, NRB, D], BF16, tag="v_sb")
        wo_b = persist.tile([P, NRB, D], BF16, tag="wob")
        ot = persist.tile([P, NRB, N], BF16, tag="ot")

        with tc.tile_pool(name="tpool", bufs=1) as tpool, \
             tc.tile_pool(name="work2", bufs=3) as work2, \
             tc.tile_pool(name="work3", bufs=1) as work3, \
             tc.tile_pool(name="work1", bufs=2) as work1, \
             tc.tile_pool(name="psum1", bufs=3, space="PSUM") as psum1:
            # phase 2: ln fields (ACT Square/Ln + DVE sub/clamp)
            for rb in range(NRB):
                for (ci, cbc, ncol, l2bc) in ((0, cxbc, negcx, l2wbc),
                                              (1, cybc, negcy, l2hbc)):
                    d2 = work2.tile([P, N], F32, tag="geo_d2")
                    nc.scalar.activation(d2[:], cbc, AF.Square,
                                         bias=ncol[:, rb:rb + 1])
                    l2t = work2.tile([P, N], F32, tag="geo_l2")
                    nc.scalar.activation(l2t[:], d2[:], AF.Ln)
                    g_ = work2.tile([P, N], F32, tag="geo_g")
                    nc.vector.tensor_tensor(g_[:], l2t[:], l2bc, ALU.subtract)
                    nc.vector.tensor_scalar_max(dxy2[:, rb, ci, :], g_[:], C2)

            # phase 3: dw/dh banks (early; DVE idle at start)
            for (pcol, bank) in ((pcolm, bankM), (pcoln, bankN)):
                t_ = work3.tile([64, N], F32, tag="bk_t")
                nc.vector.tensor_scalar(t_[:32, :], l2wbc[:32, :],
                                        acol[:32, :], pcol[:32, :],
                                        ALU.mult, ALU.add)
                nc.vector.tensor_scalar(t_[32:, :], l2hbc[32:64, :],
                                        acol[32:, :], pcol[32:, :],
                                        ALU.mult, ALU.add)
                r_ = work3.tile([64, N], F32, tag="bk_r")
                nc.vector.tensor_scalar(r_[:], t_[:], MAGIC, -MAGIC,
                                        ALU.add, ALU.add)
                f_ = work3.tile([64, N], F32, tag="bk_f")
                nc.vector.tensor_tensor(f_[:], t_[:], r_[:], ALU.subtract)
                nc.scalar.activation(bank[:], f_[:], AF.Sin, scale=TWO_PI)

            # phase 1: transposes + projections
            xqTb = tpool.tile([P, NRB, N], BF16, tag="xqTb")
            xkTb = tpool.tile([P, NRB, N], BF16, tag="xkTb")
            xvTb = tpool.tile([P, NRB, N], BF16, tag="xvTb")
            wq_b = tpool.tile([P, NRB, D], BF16, tag="wqb")
            wk_b = tpool.tile([P, NRB, D], BF16, tag="wkb")
            wv_b = tpool.tile([P, NRB, D], BF16, tag="wvb")

            kk = 0
            for (xs, dstb) in ((xq_sb, xqTb), (xk_sb, xkTb), (xv_sb, xvTb)):
                for rb in range(NRB):
                    for cb in range(NRB):
                        tp = psum1.tile([P, P], F32, tag="tp")
                        nc.tensor.transpose(tp[:], xs[:, rb, cb * P:(cb + 1) * P],
                                            ident)
                        dst = dstb[:, cb, rb * P:(rb + 1) * P]
                        if kk % 2 == 0:
                            nc.vector.tensor_copy(dst, tp[:])
                        else:
                            nc.scalar.activation(dst, tp[:], AF.Identity)
                        kk += 1
            for (wf, wb_) in ((wq_f, wq_b), (wk_f, wk_b), (wv_f, wv_b),
                              (wo_f, wo_b)):
                nc.gpsimd.tensor_copy(wb_[:], wf[:])

            for (wb_, xb, dstT, bcol, scl) in (
                    (wq_b, xqTb, qT, bqcol, 1.0),
                    (wk_b, xkTb, kTt, bkcol, 0.125)):
                for ob in range(NRB):
                    ps = psum1.tile([P, N], F32, tag="projps")
                    for kb in range(NRB):
                        nc.tensor.matmul(ps[:],
                                         wb_[:, kb, ob * P:(ob + 1) * P],
                                         xb[:, kb, :],
                                         start=(kb == 0),
                                         stop=(kb == NRB - 1))
                    nc.scalar.activation(dstT[:, ob, :], ps[:], AF.Identity,
                                         scale=scl, bias=bcol[:, ob:ob + 1])
            for mb in range(NRB):
                ps = psum1.tile([P, D], F32, tag="projps")
                for kb in range(NRB):
                    nc.tensor.matmul(ps[:], xvTb[:, kb, mb * P:(mb + 1) * P],
                                     wv_b[:, kb, :],
                                     start=(kb == 0), stop=(kb == NRB - 1))
                nc.scalar.activation(v_sb[:, mb, :], ps[:], AF.Identity)

        # ---------------- phase 4: wg ----------------
        wgdT = persist.tile([P, H, NRB, N], BF16, tag="wgdT")
        with tc.tile_pool(name="work4", bufs=2) as work4, \
             tc.tile_pool(name="psum_u", bufs=2, space="PSUM") as psum_u, \
             tc.tile_pool(name="psum_wg", bufs=3, space="PSUM") as psum_wg:
            for rb in range(NRB):
                wgd_il = work4.tile([P, NG, N], BF16, tag="wgd_il")
                for g in range(NG):
                    off = 64 * (g // 4)
                    qq = g % 4
                    ups = psum_u.tile([P, 2, N], F32, tag="ups")
                    for ci in range(2):
                        nc.tensor.matmul(ups[:, ci, :],
                                         cR[off:off + 64, qq * P:(qq + 1) * P],
                                         dxy2[off:off + 64, rb, ci, :],
                                         start=True, stop=True)
                    rr = work4.tile([P, 2, N], F32, tag="fold_r")
                    nc.vector.tensor_scalar(rr[:], ups[:], MAGIC, -MAGIC,
                                            ALU.add, ALU.add)
                    ff = work4.tile([P, 2, N], F32, tag="fold_f")
                    nc.vector.tensor_tensor(ff[:], ups[:], rr[:],
                                            ALU.subtract)
                    habs = work4.tile([P, 2, N], F32, tag="habs")
                    nc.scalar.activation(habs[:], ff[:], AF.Abs)
                    sin4 = work4.tile([P, 4, N], BF16, tag="sin4")
                    nc.scalar.activation(sin4[:, 0:2, :], ff[:], AF.Sin,
                                         scale=TWO_PI)
                    # cos(2pi f) = sin(pi/2 - 2pi |f|)
                    nc.scalar.activation(sin4[:, 2:4, :], habs[:], AF.Sin,
                                         scale=-TWO_PI, bias=halfpi[:])
                    lhs_wh = work4.tile([64, P], BF16, tag="lhs_wh")
                    mbase = rb * P + g * GM
                    nc.gpsimd.tensor_tensor(
                        lhs_wh[:].rearrange("k (h m) -> k h m", h=H),
                        w1e_b[:].rearrange("k (h m) -> k h m", h=H),
                        bankM[:, mbase:mbase + GM][:, None, :]
                            .to_broadcast((64, H, GM)),
                        ALU.mult)
                    wgp = psum_wg.tile([P, N], F32, tag="wgp")
                    for c in range(4):
                        nc.tensor.matmul(wgp[:], wblk_b[:, c, :],
                                         sin4[:, c, :],
                                         start=(c == 0), stop=False)
                    nc.tensor.matmul(wgp[:], lhs_wh[:], bankN[:],
                                     start=False, stop=True)
                    # wgd = max(wg + bG, 1e-6) - 1 = max(wg + (bG-1), 1e-6-1)
                    nc.vector.tensor_scalar(wgd_il[:, g, :], wgp[:],
                                            bgm1[:], 1e-6 - 1.0,
                                            ALU.add, ALU.max)
                # h-major -> m-major permutation via DRAM bounce
                # (SBUF->SBUF DMA honors only one partition dim on HW, and
                # SBUF-side APs may carry only one partition dim, so the
                # write side goes per (rb, h)).
                for h in range(H):
                    nc.sync.dma_start(
                        wgd_dram[rb, h]
                            .rearrange("(g ml) n -> ml g n", g=NG),
                        wgd_il[h * GM:(h + 1) * GM, :, :])
                nc.sync.dma_start(
                    wgdT[:, :, rb, :],
                    wgd_dram[rb].rearrange("h p n -> p h n"))

        # ---------------- phase 5: attention ----------------
        with tc.tile_pool(name="work5", bufs=3) as work5, \
             tc.tile_pool(name="psum5", bufs=2, space="PSUM") as psum5, \
             tc.tile_pool(name="psum_s", bufs=1, space="PSUM") as psum_s, \
             tc.tile_pool(name="psum_av", bufs=2, space="PSUM") as psum_av, \
             tc.tile_pool(name="psum_rb", bufs=1, space="PSUM") as psum_rb, \
             tc.tile_pool(name="dbgpool", bufs=1) as dbgpool:

            objpair = persist.tile([P, NRB, N], BF16, tag="objpair")
            for rb in range(NRB):
                nc.vector.tensor_scalar(objpair[:, rb, :], objbc[:],
                                        ocol[:, rb:rb + 1], None, ALU.mult)
            # head PAIRS (2k, 2k+1) share kT/qT block ob=k at offsets 0/64
            for ob in range(H // 2):
                h0 = 2 * ob
                av = psum_av.tile([P, N], F32, tag="avps")
                sbank = psum_s.tile([H, N], F32, tag="sbank")
                for rb in range(NRB):
                    st2 = psum5.tile([P, 2, N], F32, tag="stps")
                    for hi in range(2):
                        po = hi * DK
                        nc.tensor.matmul(
                            st2[:, hi, :],
                            kTt[po:po + DK, ob, rb * P:(rb + 1) * P],
                            qT[po:po + DK, ob, :], start=True, stop=True)
                    e_ = work5.tile([P, 2, N], BF16, tag="e_t")
                    nc.scalar.activation(e_[:], st2[:], AF.Exp,
                                         bias=mcol[:, rb:rb + 1])
                    e1 = work5.tile([P, 2, N], BF16, tag="e1_t")
                    e1_eng = nc.gpsimd if (ob + rb) % 2 == 0 else nc.vector
                    e1_eng.tensor_tensor(
                        e1[:], e_[:],
                        objpair[:, rb, None, :].to_broadcast((P, 2, N)),
                        ALU.mult)
                    e2 = work5.tile([P, 2, N], BF16, tag="e2_t")
                    nc.vector.tensor_tensor(e2[:], e1[:],
                                            wgdT[:, h0:h0 + 2, rb, :],
                                            ALU.mult)
                    tt_ = work5.tile([P, 2, N], BF16, tag="tt_t")
                    nc.vector.tensor_tensor(tt_[:], e_[:], e2[:], ALU.add)
                    for hi in range(2):
                        po = hi * DK
                        nc.tensor.matmul(sbank[:],
                                         oh8_b[:, (h0 + hi) * H:
                                               (h0 + hi + 1) * H],
                                         tt_[:, hi, :],
                                         start=(rb == 0 and hi == 0),
                                         stop=(rb == NRB - 1 and hi == 1),
                                         skip_group_check=True)
                        nc.tensor.matmul(av[po:po + DK, :],
                                         v_sb[:, rb,
                                              (h0 + hi) * DK:(h0 + hi + 1) * DK],
                                         tt_[:, hi, :], start=(rb == 0),
                                         stop=(rb == NRB - 1),
                                         skip_group_check=True)
                if debug and ob == 0:
                    dbg_sb = nc.declare_dram_parameter(
                        "dbg_sbank", [H, N], F32, isOutput=True)
                    sb_c = dbgpool.tile([H, N], F32, tag="dbg_sbc")
                    nc.vector.tensor_copy(sb_c[:], sbank[:])
                    nc.sync.dma_start(dbg_sb[:], sb_c[:])
                    dbg_av = nc.declare_dram_parameter(
                        "dbg_av", [P, N], F32, isOutput=True)
                    av_c = dbgpool.tile([P, N], F32, tag="dbg_avc")
                    nc.vector.tensor_copy(av_c[:], av[:])
                    nc.sync.dma_start(dbg_av[:], av_c[:])
                sb2 = work5.tile([H, N], F32, tag="sb2")
                nc.vector.tensor_scalar(sb2[:], sbank[:],
                                        zcol[0:8, ob:ob + 1], None, ALU.add)
                rs = work5.tile([H, N], F32, tag="rs")
                nc.vector.reciprocal(rs[:], sb2[:])
                rrb = psum_rb.tile([P, N], F32, tag="rrb")
                nc.tensor.matmul(rrb[:], cF[0:8, OFF_REPL + ob * P:
                                            OFF_REPL + (ob + 1) * P],
                                 rs[0:8, :], start=True, stop=True)
                if debug and ob == 0:
                    dbg_rs = nc.declare_dram_parameter(
                        "dbg_rs", [H, N], F32, isOutput=True)
                    nc.sync.dma_start(dbg_rs[:], rs[:])
                    dbg_rrb = nc.declare_dram_parameter(
                        "dbg_rrb", [P, N], F32, isOutput=True)
                    rrb_c = dbgpool.tile([P, N], F32, tag="dbg_rrbc")
                    nc.vector.tensor_copy(rrb_c[:], rrb[:])
                    nc.sync.dma_start(dbg_rrb[:], rrb_c[:])
                rrb_sb = work5.tile([P, N], F32, tag="rrb_sb")
                nc.scalar.activation(rrb_sb[:], rrb[:], AF.Identity)
                nc.vector.tensor_tensor(ot[:, ob, :], av[:], rrb_sb[:],
                                        ALU.mult)

        if debug:
            dbg_dxy2 = nc.declare_dram_parameter(
                "dbg_dxy2", [P, NRB, 2, N], F32R, isOutput=True)
            nc.sync.dma_start(dbg_dxy2[:], dxy2[:])
            dbg_wgdT = nc.declare_dram_parameter(
                "dbg_wgdT", [P, H, NRB, N], BF16, isOutput=True)
            nc.sync.dma_start(dbg_wgdT[:], wgdT[:])
            dbg_qT = nc.declare_dram_parameter(
                "dbg_qT", [P, NRB, N], BF16, isOutput=True)
            nc.sync.dma_start(dbg_qT[:], qT[:])
            dbg_kT = nc.declare_dram_parameter(
                "dbg_kT", [P, NRB, N], BF16, isOutput=True)
            nc.sync.dma_start(dbg_kT[:], kTt[:])
            dbg_v = nc.declare_dram_parameter(
                "dbg_v", [P, NRB, D], BF16, isOutput=True)
            nc.sync.dma_start(dbg_v[:], v_sb[:])
            dbg_ot = nc.declare_dram_parameter(
                "dbg_ot", [P, NRB, N], BF16, isOutput=True)
            nc.sync.dma_start(dbg_ot[:], ot[:])

        # final projection: out[n, d]
        with tc.tile_pool(name="work6", bufs=2) as work6, \
             tc.tile_pool(name="psum6", bufs=2, space="PSUM") as psum6:
            for r in range(NRB):
                ps = psum6.tile([P, D], F32, tag="fps")
                for kt in range(NRB):
                    nc.tensor.matmul(ps[:], ot[:, kt, r * P:(r + 1) * P],
                                     wo_b[:, kt, :],
                                     start=(kt == 0), stop=(kt == NRB - 1))
                fo = work6.tile([P, D], F32, tag="fo")
                nc.vector.tensor_tensor(fo[:], ps[:], bobc, ALU.add)
                nc.sync.dma_start(out[r * P:(r + 1) * P, :], fo[:])

    if fix_waits:
        _split_multi_waits(nc)
    return nc


_NC_CACHE = {}


def kernel(**inputs):
    in_maps = _host_prep(inputs)
    if "nc" not in _NC_CACHE:
        _NC_CACHE["nc"] = build_nc()
    nc = _NC_CACHE["nc"]
    res = run_bass_kernel_spmd(nc, in_maps, list(range(B)))
    out = np.stack([res.results[b]["out"] for b in range(B)], axis=0)
    return out.astype(np.float32)


if __name__ == "__main__":
    print("kernel module ok")


# revision 38
# speedup vs baseline: 1.0716x; 1.0041x over previous
"""Trainium2 Bass kernel for BoxMultiHeadedAttention (B=8, N=512, D=512, H=8).

Sharding: data-parallel over batch — each of the 8 NeuronCores computes one
batch element end-to-end; weights replicated; no collectives.

Per-core algorithm (transposed-attention layout [m(part), n(free)]):
  * q/k/v projections on PE (bf16) from PE-transposed inputs; PSUM evictions
    on ACT (Identity with scale/bias folds the q/k biases; 1/8 folded into k).
  * geometry wg:
      - dx/dy: ln fields via ACT Square(bias=-c)/Ln + DVE sub/clamp; phase
        fractions t = (alpha_j/4pi)*dx2 by f32r selector matmuls on PE;
        magic-round fold on DVE (rr, ff) + |f|-1/4 on Pool; ONE stacked ACT
        Sin pass yields [sin(2pi f); -cos(2pi f)] with the cos sign folded
        into the WBLK weights; WG contraction on PE (bf16).
      - dw/dh: exactly separable -> rank-64 PE contraction of per-box
        sin/cos banks.
      - h-major -> m-major partition permutation via 8 merged strided DMAs
        per row-block (3-dim APs).
  * exp-domain softmax: T = E*(1 + obj*wgd); E on ACT, obj-mult on Pool,
    wgd-mult/add on DVE; row sums via PE one-hot matmul; 1/s broadcast via
    PE selector matmul (f32r); final linear on PE.
  * bv is folded into bo on the host (bo' = bo + bv @ Wo).
"""
import math
import numpy as np
from contextlib import ExitStack

import concourse.bass as bass
import concourse.mybir as mybir
import concourse.tile as tile
from concourse.bass_utils import run_bass_kernel_spmd

F32 = mybir.dt.float32
F32R = mybir.dt.float32r
BF16 = mybir.dt.bfloat16
AF = mybir.ActivationFunctionType
ALU = mybir.AluOpType

B, N, D, H = 8, 512, 512, 8
DK = D // H
P = 128
NRB = N // P
NG = 8
GM = 16
WAVE_LEN = 1000.0
MAGIC = 12582912.0
C2 = float(2.0 * math.log(0.001))
ESHIFT = -6.0
TWO_PI = float(2.0 * math.pi)

_alphas = (100.0 / (WAVE_LEN ** (np.arange(8) / 8.0))).astype(np.float64)

# const-blob column offsets (f32 blob)
OFF_IDENT = 0          # [P, 128]
OFF_OH8 = 128          # [P, 64]  col = h*8 + c
OFF_WBLK = 192         # [P, 4*128]
OFF_W1E = 704          # [64, 128]
OFF_BG = 832           # [P, 1]
OFF_ACOL = 833         # [64, 1]
OFF_PCOLM = 834        # [64, 1]
OFF_PCOLN = 835        # [64, 1]
OFF_REPL = 836         # [8, 4*128]  col = ob*128 + p; 1 iff h == ob*2+p//64
CBLOB_W = 1348
# f32r blob: selap [P, 4*128] (col q*128+c)
CBLOBR_W = 512


def _split_multi_waits(nc):
    """walrus here accepts only ONE sync-wait per ISA instruction; hoist
    extras onto NoOps inserted before the offending instruction."""
    n_fix = 0
    for blk in nc.main_func.blocks:
        insts = list(blk.instructions)
        out, dirty = [], False
        for inst in insts:
            si = inst.sync_info
            waits = list(si.on_wait) if si is not None else []
            if len(waits) > 1:
                for kk, w in enumerate(waits[:-1]):
                    out.append(mybir.InstNoOp(
                        name=f"I-waitfix-{n_fix}-{kk}", engine=inst.engine,
                        sync_info=mybir.SyncInfo(on_wait=[w], on_update=[])))
                inst.sync_info = mybir.SyncInfo(
                    on_wait=[waits[-1]], on_update=list(si.on_update))
                n_fix += 1
                dirty = True
            out.append(inst)
        if dirty:
            blk.instructions = out
    return n_fix


def _build_cblob(WG, bG):
    cb = np.zeros((P, CBLOB_W), dtype=np.float32)
    cb[:, OFF_IDENT:OFF_IDENT + P] = np.eye(P, dtype=np.float32)
    # one-hot columns for row sums: OH8[p, h*8+c] = 1 iff c == h
    for h in range(H):
        cb[:, OFF_OH8 + h * H + h] = 1.0
    # WBLK: direct sin/cos weights.  sin4 layout: [:,0,:]=sin dx,
    # [:,1,:]=sin dy, [:,2,:]=cos dx, [:,3,:]=cos dy.
    gmap = [lambda j: j, lambda j: 8 + j, lambda j: 32 + j, lambda j: 40 + j]
    gscl = [1.0, 1.0, 1.0, 1.0]
    for c in range(4):
        for ml in range(GM):
            for j in range(8):
                for h in range(H):
                    cb[ml * 8 + j, OFF_WBLK + c * P + h * GM + ml] = \
                        gscl[c] * WG[h, gmap[c](j)]
    # dw/dh rank-64 weights (angle-addition banks; unchanged from the
    # half-angle formulation since the banks encode sin/cos via phase
    # offsets in PCOL)
    w1 = np.zeros((64, H), np.float32)
    acol = np.zeros((64,), np.float32)
    pcol_m = np.zeros((64,), np.float32)
    pcol_n = np.zeros((64,), np.float32)
    for f in range(2):
        for j in range(8):
            gs = 16 + 8 * f + j
            gc = 48 + 8 * f + j
            a = _alphas[j] / (4.0 * math.pi)
            for t in range(4):
                k = (f * 8 + j) * 4 + t
                acol[k] = a
                pcol_m[k] = 0.25 if t in (0, 2) else 0.0
                if t == 0:
                    pcol_n[k] = 0.0; w1[k] = WG[:, gs]
                elif t == 1:
                    pcol_n[k] = 0.75; w1[k] = WG[:, gs]   # -cos -> +pi
                elif t == 2:
                    pcol_n[k] = 0.25; w1[k] = WG[:, gc]
                else:
                    pcol_n[k] = 0.0; w1[k] = WG[:, gc]
    cb[0:64, OFF_W1E:OFF_W1E + P] = np.repeat(w1, GM, axis=1)
    cb[:, OFF_BG] = np.repeat(bG.astype(np.float64), GM).astype(np.float32)
    cb[0:64, OFF_ACOL] = acol
    cb[0:64, OFF_PCOLM] = pcol_m
    cb[0:64, OFF_PCOLN] = pcol_n
    for ob in range(H // 2):
        for hi in range(2):
            cb[ob * 2 + hi, OFF_REPL + ob * P + hi * DK:
               OFF_REPL + ob * P + (hi + 1) * DK] = 1.0
    return cb


def _build_cblobr():
    cr = np.zeros((P, CBLOBR_W), dtype=np.float32)
    # SELAP[64*W + q*16 + ml, q*128 + ml*8 + j] = alpha_j/(4pi)
    for W in range(2):
        for q in range(4):
            for ml in range(GM):
                for j in range(8):
                    cr[64 * W + q * GM + ml, q * P + ml * 8 + j] = \
                        _alphas[j] / (4.0 * math.pi)
    return cr


def _host_prep(inputs):
    q = np.asarray(inputs["input_query"], np.float32)
    k = np.asarray(inputs["input_key"], np.float32)
    v = np.asarray(inputs["input_value"], np.float32)
    box = np.asarray(inputs["input_box"], np.float32)
    mask = np.asarray(inputs["mask"])
    nobj = np.asarray(inputs["not_objects"])
    WG = np.asarray(inputs["WG"], np.float32)
    bG = np.asarray(inputs["bG"], np.float32)
    Wo = np.asarray(inputs["Wo"], np.float32)
    bo = np.asarray(inputs["bo"], np.float32)
    bv = np.asarray(inputs["bv"], np.float32)

    x_min, y_min, x_max, y_max = [box[..., i] for i in range(4)]
    cx = (x_min + x_max) * 0.5
    cy = (y_min + y_max) * 0.5
    ww = x_max - x_min + 1.0
    hh = y_max - y_min + 1.0
    l2w = (2.0 * np.log(ww)).astype(np.float32)
    l2h = (2.0 * np.log(hh)).astype(np.float32)

    maskcol = (np.where(mask == 0, -1e9, 0.0) + ESHIFT).astype(np.float32)
    obj = (1.0 - nobj.astype(np.float32)).astype(np.float32)
    borow = (bo.astype(np.float64) + bv.astype(np.float64)
             @ Wo.astype(np.float64)).astype(np.float32)

    def col(a):  # [N] -> [P, NRB]
        return a.reshape(NRB, P).T

    shared = {
        "Wq": np.asarray(inputs["Wq"], np.float32),
        "Wk": np.asarray(inputs["Wk"], np.float32),
        "Wv": np.asarray(inputs["Wv"], np.float32),
        "Wo": Wo,
        "CBLOB": _build_cblob(WG, bG),
        "CBLOBR": _build_cblobr(),
    }
    bqc = col(np.asarray(inputs["bq"], np.float32))
    bkc = col(np.asarray(inputs["bk"], np.float32))
    in_maps = []
    for b in range(B):
        cols = np.zeros((P, 28), np.float32)
        for ob in range(4):
            cols[:, 24 + ob] = 1.0
            cols[2 * ob, 24 + ob] = 0.0
            cols[2 * ob + 1, 24 + ob] = 0.0
        cols[:, 0:4] = col(maskcol[b])
        cols[:, 4:8] = bqc
        cols[:, 8:12] = bkc
        cols[:, 12:16] = -col(cx[b])
        cols[:, 16:20] = -col(cy[b])
        cols[:, 20:24] = col(obj[b])
        rows = np.stack([cx[b], cy[b], l2w[b], l2h[b], obj[b], borow], 0)
        m = dict(shared)
        m.update({
            "xq": q[b].copy(), "xk": k[b].copy(), "xv": v[b].copy(),
            "COLS": cols, "ROWS": rows.astype(np.float32).copy(),
        })
        in_maps.append(m)
    return in_maps


def build_nc(fix_waits=True, perm_merge=True, debug=False):
    nc = bass.Bass()

    def dp(name, shape, dt=F32):
        return nc.declare_dram_parameter(name, list(shape), dt, isOutput=False)

    xq = dp("xq", (N, D)); xk = dp("xk", (N, D)); xv = dp("xv", (N, D))
    Wq = dp("Wq", (D, D)); Wk = dp("Wk", (D, D)); Wv = dp("Wv", (D, D))
    Wo = dp("Wo", (D, D))
    CBLOB = dp("CBLOB", (P, CBLOB_W))
    CBLOBR = dp("CBLOBR", (P, CBLOBR_W), F32R)
    COLS = dp("COLS", (P, 28))
    ROWS = dp("ROWS", (6, N))
    out = nc.declare_dram_parameter("out", [N, D], F32, isOutput=True)
    wgd_dram = nc.dram_tensor("wgd_scratch", [NRB, H, P, N], BF16)

    with ExitStack() as ctx:
        tc = ctx.enter_context(tile.TileContext(nc))
        const = ctx.enter_context(tc.tile_pool(name="const", bufs=1))
        persist = ctx.enter_context(tc.tile_pool(name="persist", bufs=1))

        # ---- const loads (DMA order favors phase-1/2 start) ----
        loadp = ctx.enter_context(tc.tile_pool(name="loadp", bufs=1))
        xq_sb = loadp.tile([P, NRB, D], F32, tag="xq_sb")
        nc.sync.dma_start(xq_sb[:], xq.rearrange("(rb p) d -> p rb d", p=P))
        cF = const.tile([P, CBLOB_W], F32, tag="cF")
        # ident first: unblocks the PE transposes ~4us earlier than the
        # full blob would
        nc.sync.dma_start(cF[:, 0:P], CBLOB[:, 0:P])
        cols_t = const.tile([P, 28], F32, tag="cols")
        nc.sync.dma_start(cols_t[:], COLS[:])
        rows_t = const.tile([P, 6, N], F32, tag="rows")
        nc.sync.dma_start(rows_t[:, 0:4, :],
                          ROWS[None, 0:4, :].to_broadcast((P, 4, N)))
        nc.sync.dma_start(cF[:, P:], CBLOB[:, P:])
        nc.sync.dma_start(rows_t[:, 4:6, :],
                          ROWS[None, 4:6, :].to_broadcast((P, 2, N)))
        wq_f = loadp.tile([P, NRB, D], F32, tag="wq_f")
        nc.sync.dma_start(wq_f[:], Wq.rearrange("(kb p) d -> p kb d", p=P))
        xk_sb = loadp.tile([P, NRB, D], F32, tag="xk_sb")
        nc.sync.dma_start(xk_sb[:], xk.rearrange("(rb p) d -> p rb d", p=P))
        xv_sb = loadp.tile([P, NRB, D], F32, tag="xv_sb")
        nc.sync.dma_start(xv_sb[:], xv.rearrange("(rb p) d -> p rb d", p=P))
        wk_f = loadp.tile([P, NRB, D], F32, tag="wk_f")
        nc.sync.dma_start(wk_f[:], Wk.rearrange("(kb p) d -> p kb d", p=P))
        wv_f = loadp.tile([P, NRB, D], F32, tag="wv_f")
        nc.sync.dma_start(wv_f[:], Wv.rearrange("(kb p) d -> p kb d", p=P))
        cR = const.tile([P, CBLOBR_W], F32R, tag="cR")
        nc.sync.dma_start(cR[:], CBLOBR[:])
        wo_f = loadp.tile([P, NRB, D], F32, tag="wo_f")
        nc.sync.dma_start(wo_f[:], Wo.rearrange("(kb p) d -> p kb d", p=P))

        ident = cF[:, OFF_IDENT:OFF_IDENT + P]
        mcol = cols_t[:, 0:4]
        bqcol = cols_t[:, 4:8]
        bkcol = cols_t[:, 8:12]
        negcx = cols_t[:, 12:16]
        negcy = cols_t[:, 16:20]
        ocol = cols_t[:, 20:24]
        zcol = cols_t[:, 24:28]
        cxbc = rows_t[:, 0, :]
        cybc = rows_t[:, 1, :]
        l2wbc = rows_t[:, 2, :]
        l2hbc = rows_t[:, 3, :]
        objbc_f = rows_t[:, 4, :]
        bobc = rows_t[:, 5, :]

        # small const casts / derived
        oh8_b = const.tile([P, H * H], BF16, tag="oh8b")
        nc.vector.tensor_copy(oh8_b[:], cF[:, OFF_OH8:OFF_OH8 + H * H])
        wblk_b = const.tile([P, 4, P], BF16, tag="wblkb")
        for c in range(4):
            nc.gpsimd.tensor_copy(wblk_b[:, c, :],
                                  cF[:, OFF_WBLK + c * P:OFF_WBLK + (c + 1) * P])
        w1e_b = const.tile([64, P], BF16, tag="w1eb")
        nc.gpsimd.tensor_copy(w1e_b[:], cF[0:64, OFF_W1E:OFF_W1E + P])
        objbc = const.tile([P, N], BF16, tag="objbc")
        nc.gpsimd.tensor_copy(objbc[:], objbc_f[:])
        halfpi = const.tile([P, 1], F32, tag="halfpi")
        nc.vector.memset(halfpi[:], float(math.pi / 2.0))
        bgm1 = const.tile([P, 1], F32, tag="bgm1")
        nc.vector.tensor_scalar(bgm1[:], cF[:, OFF_BG:OFF_BG + 1], -1.0, None,
                                ALU.add)
        acol = cF[0:64, OFF_ACOL:OFF_ACOL + 1]
        pcolm = cF[0:64, OFF_PCOLM:OFF_PCOLM + 1]
        pcoln = cF[0:64, OFF_PCOLN:OFF_PCOLN + 1]

        # ---------------- phases 1+2 (shared scope so they overlap) -------
        dxy2 = persist.tile([P, NRB, 2, N], F32R, tag="dxy2")
        bankM = persist.tile([64, N], BF16, tag="bankM")
        bankN = persist.tile([64, N], BF16, tag="bankN")
        qT = persist.tile([P, NRB, N], BF16, tag="qT")
        kTt = persist.tile([P, NRB, N], BF16, tag="kT")
        v_sb = persist.tile([P, NRB, D], BF16, tag="v_sb")
        wo_b = persist.tile([P, NRB, D], BF16, tag="wob")
        ot = persist.tile([P, NRB, N], BF16, tag="ot")

        with tc.tile_pool(name="tpool", bufs=1) as tpool, \
             tc.tile_pool(name="work2", bufs=3) as work2, \
             tc.tile_pool(name="work3", bufs=1) as work3, \
             tc.tile_pool(name="work1", bufs=2) as work1, \
             tc.tile_pool(name="psum1", bufs=4, space="PSUM") as psum1:
            # phase 2: ln fields (ACT Square/Ln + DVE sub/clamp)
            for rb in range(NRB):
                for (ci, cbc, ncol, l2bc) in ((0, cxbc, negcx, l2wbc),
                                              (1, cybc, negcy, l2hbc)):
                    d2 = work2.tile([P, N], F32, tag="geo_d2")
                    nc.scalar.activation(d2[:], cbc, AF.Square,
                                         bias=ncol[:, rb:rb + 1])
                    l2t = work2.tile([P, N], F32, tag="geo_l2")
                    nc.scalar.activation(l2t[:], d2[:], AF.Ln)
                    g_ = work2.tile([P, N], F32, tag="geo_g")
                    nc.vector.tensor_tensor(g_[:], l2t[:], l2bc, ALU.subtract)
                    nc.vector.tensor_scalar_max(dxy2[:, rb, ci, :], g_[:], C2)

            # phase 3: dw/dh banks (early; DVE idle at start)
            for (pcol, bank) in ((pcolm, bankM), (pcoln, bankN)):
                t_ = work3.tile([64, N], F32, tag="bk_t")
                nc.vector.tensor_scalar(t_[:32, :], l2wbc[:32, :],
                                        acol[:32, :], pcol[:32, :],
                                        ALU.mult, ALU.add)
                nc.vector.tensor_scalar(t_[32:, :], l2hbc[32:64, :],
                                        acol[32:, :], pcol[32:, :],
                                        ALU.mult, ALU.add)
                r_ = work3.tile([64, N], F32, tag="bk_r")
                nc.vector.tensor_scalar(r_[:], t_[:], MAGIC, -MAGIC,
                                        ALU.add, ALU.add)
                f_ = work3.tile([64, N], F32, tag="bk_f")
                nc.vector.tensor_tensor(f_[:], t_[:], r_[:], ALU.subtract)
                nc.scalar.activation(bank[:], f_[:], AF.Sin, scale=TWO_PI)

            # phase 1: transposes + projections
            xqTb = tpool.tile([P, NRB, N], BF16, tag="xqTb")
            xkTb = tpool.tile([P, NRB, N], BF16, tag="xkTb")
            xvTb = tpool.tile([P, NRB, N], BF16, tag="xvTb")
            wq_b = tpool.tile([P, NRB, D], BF16, tag="wqb")
            wk_b = tpool.tile([P, NRB, D], BF16, tag="wkb")
            wv_b = tpool.tile([P, NRB, D], BF16, tag="wvb")

            kk = 0
            for (xs, dstb) in ((xq_sb, xqTb), (xk_sb, xkTb), (xv_sb, xvTb)):
                for rb in range(NRB):
                    for cb in range(NRB):
                        tp = psum1.tile([P, P], F32, tag="tp")
                        nc.tensor.transpose(tp[:], xs[:, rb, cb * P:(cb + 1) * P],
                                            ident)
                        dst = dstb[:, cb, rb * P:(rb + 1) * P]
                        if kk % 2 == 0:
                            nc.vector.tensor_copy(dst, tp[:])
                        else:
                            nc.scalar.activation(dst, tp[:], AF.Identity)
                        kk += 1
            for (wf, wb_) in ((wq_f, wq_b), (wk_f, wk_b), (wv_f, wv_b),
                              (wo_f, wo_b)):
                nc.gpsimd.tensor_copy(wb_[:], wf[:])

            for (wb_, xb, dstT, bcol, scl) in (
                    (wq_b, xqTb, qT, bqcol, 1.0),
                    (wk_b, xkTb, kTt, bkcol, 0.125)):
                for ob in range(NRB):
                    ps = psum1.tile([P, N], F32, tag="projps")
                    for kb in range(NRB):
                        nc.tensor.matmul(ps[:],
                                         wb_[:, kb, ob * P:(ob + 1) * P],
                                         xb[:, kb, :],
                                         start=(kb == 0),
                                         stop=(kb == NRB - 1))
                    nc.scalar.activation(dstT[:, ob, :], ps[:], AF.Identity,
                                         scale=scl, bias=bcol[:, ob:ob + 1])
            for mb in range(NRB):
                ps = psum1.tile([P, D], F32, tag="projps")
                for kb in range(NRB):
                    nc.tensor.matmul(ps[:], xvTb[:, kb, mb * P:(mb + 1) * P],
                                     wv_b[:, kb, :],
                                     start=(kb == 0), stop=(kb == NRB - 1))
                nc.scalar.activation(v_sb[:, mb, :], ps[:], AF.Identity)

        # ---------------- phase 4: wg ----------------
        wgdT = persist.tile([P, H, NRB, N], BF16, tag="wgdT")
        with tc.tile_pool(name="work4", bufs=2) as work4, \
             tc.tile_pool(name="psum_u", bufs=2, space="PSUM") as psum_u, \
             tc.tile_pool(name="psum_wg", bufs=3, space="PSUM") as psum_wg:
            for rb in range(NRB):
                wgd_il = work4.tile([P, NG, N], BF16, tag="wgd_il")
                for g in range(NG):
                    off = 64 * (g // 4)
                    qq = g % 4
                    ups = psum_u.tile([P, 2, N], F32, tag="ups")
                    for ci in range(2):
                        nc.tensor.matmul(ups[:, ci, :],
                                         cR[off:off + 64, qq * P:(qq + 1) * P],
                                         dxy2[off:off + 64, rb, ci, :],
                                         start=True, stop=True)
                    rr = work4.tile([P, 2, N], F32, tag="fold_r")
                    nc.vector.tensor_scalar(rr[:], ups[:], MAGIC, -MAGIC,
                                            ALU.add, ALU.add)
                    ff = work4.tile([P, 2, N], F32, tag="fold_f")
                    nc.vector.tensor_tensor(ff[:], ups[:], rr[:],
                                            ALU.subtract)
                    habs = work4.tile([P, 2, N], F32, tag="habs")
                    nc.scalar.activation(habs[:], ff[:], AF.Abs)
                    sin4 = work4.tile([P, 4, N], BF16, tag="sin4")
                    nc.scalar.activation(sin4[:, 0:2, :], ff[:], AF.Sin,
                                         scale=TWO_PI)
                    # cos(2pi f) = sin(pi/2 - 2pi |f|)
                    nc.scalar.activation(sin4[:, 2:4, :], habs[:], AF.Sin,
                                         scale=-TWO_PI, bias=halfpi[:])
                    lhs_wh = work4.tile([64, P], BF16, tag="lhs_wh")
                    mbase = rb * P + g * GM
                    nc.gpsimd.tensor_tensor(
                        lhs_wh[:].rearrange("k (h m) -> k h m", h=H),
                        w1e_b[:].rearrange("k (h m) -> k h m", h=H),
                        bankM[:, mbase:mbase + GM][:, None, :]
                            .to_broadcast((64, H, GM)),
                        ALU.mult)
                    wgp = psum_wg.tile([P, N], F32, tag="wgp")
                    for c in range(4):
                        nc.tensor.matmul(wgp[:], wblk_b[:, c, :],
                                         sin4[:, c, :],
                                         start=(c == 0), stop=False)
                    nc.tensor.matmul(wgp[:], lhs_wh[:], bankN[:],
                                     start=False, stop=True)
                    # wgd = max(wg + bG, 1e-6) - 1 = max(wg + (bG-1), 1e-6-1)
                    nc.vector.tensor_scalar(wgd_il[:, g, :], wgp[:],
                                            bgm1[:], 1e-6 - 1.0,
                                            ALU.add, ALU.max)
                # h-major -> m-major permutation via DRAM bounce
                # (SBUF->SBUF DMA honors only one partition dim on HW, and
                # SBUF-side APs may carry only one partition dim, so the
                # write side goes per (rb, h)).
                for h in range(H):
                    nc.sync.dma_start(
                        wgd_dram[rb, h]
                            .rearrange("(g ml) n -> ml g n", g=NG),
                        wgd_il[h * GM:(h + 1) * GM, :, :])
                nc.sync.dma_start(
                    wgdT[:, :, rb, :],
                    wgd_dram[rb].rearrange("h p n -> p h n"))

        # ---------------- phase 5: attention ----------------
        with tc.tile_pool(name="work5", bufs=3) as work5, \
             tc.tile_pool(name="psum5", bufs=2, space="PSUM") as psum5, \
             tc.tile_pool(name="psum_s", bufs=1, space="PSUM") as psum_s, \
             tc.tile_pool(name="psum_av", bufs=2, space="PSUM") as psum_av, \
             tc.tile_pool(name="psum_rb", bufs=1, space="PSUM") as psum_rb, \
             tc.tile_pool(name="dbgpool", bufs=1) as dbgpool:

            objpair = persist.tile([P, NRB, N], BF16, tag="objpair")
            for rb in range(NRB):
                nc.vector.tensor_scalar(objpair[:, rb, :], objbc[:],
                                        ocol[:, rb:rb + 1], None, ALU.mult)
            # head PAIRS (2k, 2k+1) share kT/qT block ob=k at offsets 0/64
            for ob in range(H // 2):
                h0 = 2 * ob
                av = psum_av.tile([P, N], F32, tag="avps")
                sbank = psum_s.tile([H, N], F32, tag="sbank")
                for rb in range(NRB):
                    st2 = psum5.tile([P, 2, N], F32, tag="stps")
                    for hi in range(2):
                        po = hi * DK
                        nc.tensor.matmul(
                            st2[:, hi, :],
                            kTt[po:po + DK, ob, rb * P:(rb + 1) * P],
                            qT[po:po + DK, ob, :], start=True, stop=True)
                    e_ = work5.tile([P, 2, N], BF16, tag="e_t")
                    nc.scalar.activation(e_[:], st2[:], AF.Exp,
                                         bias=mcol[:, rb:rb + 1])
                    e1 = work5.tile([P, 2, N], BF16, tag="e1_t")
                    e1_eng = nc.gpsimd if (ob + rb) % 2 == 0 else nc.vector
                    e1_eng.tensor_tensor(
                        e1[:], e_[:],
                        objpair[:, rb, None, :].to_broadcast((P, 2, N)),
                        ALU.mult)
                    e2 = work5.tile([P, 2, N], BF16, tag="e2_t")
                    nc.vector.tensor_tensor(e2[:], e1[:],
                                            wgdT[:, h0:h0 + 2, rb, :],
                                            ALU.mult)
                    tt_ = work5.tile([P, 2, N], BF16, tag="tt_t")
                    nc.vector.tensor_tensor(tt_[:], e_[:], e2[:], ALU.add)
                    for hi in range(2):
                        po = hi * DK
                        nc.tensor.matmul(sbank[:],
                                         oh8_b[:, (h0 + hi) * H:
                                               (h0 + hi + 1) * H],
                                         tt_[:, hi, :],
                                         start=(rb == 0 and hi == 0),
                                         stop=(rb == NRB - 1 and hi == 1),
                                         skip_group_check=True)
                        nc.tensor.matmul(av[po:po + DK, :],
                                         v_sb[:, rb,
                                              (h0 + hi) * DK:(h0 + hi + 1) * DK],
                                         tt_[:, hi, :], start=(rb == 0),
                                         stop=(rb == NRB - 1),
                                         skip_group_check=True)
                if debug and ob == 0:
                    dbg_sb = nc.declare_dram_parameter(
                        "dbg_sbank", [H, N], F32, isOutput=True)
                    sb_c = dbgpool.tile([H, N], F32, tag="dbg_sbc")
                    nc.vector.tensor_copy(sb_c[:], sbank[:])
                    nc.sync.dma_start(dbg_sb[:], sb_c[:])
                    dbg_av = nc.declare_dram_parameter(
                        "dbg_av", [P, N], F32, isOutput=True)
                    av_c = dbgpool.tile([P, N], F32, tag="dbg_avc")
                    nc.vector.tensor_copy(av_c[:], av[:])
                    nc.sync.dma_start(dbg_av[:], av_c[:])
                sb2 = work5.tile([H, N], F32, tag="sb2")
                nc.vector.tensor_scalar(sb2[:], sbank[:],
                                        zcol[0:8, ob:ob + 1], None, ALU.add)
                rs = work5.tile([H, N], F32, tag="rs")
                nc.vector.reciprocal(rs[:], sb2[:])
                rrb = psum_rb.tile([P, N], F32, tag="rrb")
                nc.tensor.matmul(rrb[:], cF[0:8, OFF_REPL + ob * P:
                                            OFF_REPL + (ob + 1) * P],
                                 rs[0:8, :], start=True, stop=True)
                if debug and ob == 0:
                    dbg_rs = nc.declare_dram_parameter(
                        "dbg_rs", [H, N], F32, isOutput=True)
                    nc.sync.dma_start(dbg_rs[:], rs[:])
                    dbg_rrb = nc.declare_dram_parameter(
                        "dbg_rrb", [P, N], F32, isOutput=True)
                    rrb_c = dbgpool.tile([P, N], F32, tag="dbg_rrbc")
                    nc.vector.tensor_copy(rrb_c[:], rrb[:])
                    nc.sync.dma_start(dbg_rrb[:], rrb_c[:])
                rrb_sb = work5.tile([P, N], F32, tag="rrb_sb")
                nc.scalar.activation(rrb_sb[:], rrb[:], AF.Identity)
                nc.vector.tensor_tensor(ot[:, ob, :], av[:], rrb_sb[:],
                                        ALU.mult)

        if debug:
            dbg_dxy2 = nc.declare_dram_parameter(
                "dbg_dxy2", [P, NRB, 2, N], F32R, isOutput=True)
            nc.sync.dma_start(dbg_dxy2[:], dxy2[:])
            dbg_wgdT = nc.declare_dram_parameter(
                "dbg_wgdT", [P, H, NRB, N], BF16, isOutput=True)
            nc.sync.dma_start(dbg_wgdT[:], wgdT[:])
            dbg_qT = nc.declare_dram_parameter(
                "dbg_qT", [P, NRB, N], BF16, isOutput=True)
            nc.sync.dma_start(dbg_qT[:], qT[:])
            dbg_kT = nc.declare_dram_parameter(
                "dbg_kT", [P, NRB, N], BF16, isOutput=True)
            nc.sync.dma_start(dbg_kT[:], kTt[:])
            dbg_v = nc.declare_dram_parameter(
                "dbg_v", [P, NRB, D], BF16, isOutput=True)
            nc.sync.dma_start(dbg_v[:], v_sb[:])
            dbg_ot = nc.declare_dram_parameter(
                "dbg_ot", [P, NRB, N], BF16, isOutput=True)
            nc.sync.dma_start(dbg_ot[:], ot[:])

        # final projection: out[n, d]
        with tc.tile_pool(name="work6", bufs=2) as work6, \
             tc.tile_pool(name="psum6", bufs=2, space="PSUM") as psum6:
            for r in range(NRB):
                ps = psum6.tile([P, D], F32, tag="fps")
                for kt in range(NRB):
                    nc.tensor.matmul(ps[:], ot[:, kt, r * P:(r + 1) * P],
                                     wo_b[:, kt, :],
                                     start=(kt == 0), stop=(kt == NRB - 1))
                fo = work6.tile([P, D], F32, tag="fo")
                nc.vector.tensor_tensor(fo[:], ps[:], bobc, ALU.add)
                nc.sync.dma_start(out[r * P:(r + 1) * P, :], fo[:])

    if fix_waits:
        _split_multi_waits(nc)
    return nc


_NC_CACHE = {}


def kernel(**inputs):
    in_maps = _host_prep(inputs)
    if "nc" not in _NC_CACHE:
        _NC_CACHE["nc"] = build_nc()
    nc = _NC_CACHE["nc"]
    res = run_bass_kernel_spmd(nc, in_maps, list(range(B)))
    out = np.stack([res.results[b]["out"] for b in range(B)], axis=0)
    return out.astype(np.float32)


if __name__ == "__main__":
    print("kernel module ok")


# revision 40
# speedup vs baseline: 1.0750x; 1.0031x over previous
"""Trainium2 Bass kernel for BoxMultiHeadedAttention (B=8, N=512, D=512, H=8).

Sharding: data-parallel over batch — each of the 8 NeuronCores computes one
batch element end-to-end; weights replicated; no collectives.

Per-core algorithm (transposed-attention layout [m(part), n(free)]):
  * q/k/v projections on PE (bf16) from PE-transposed inputs; PSUM evictions
    on ACT (Identity with scale/bias folds the q/k biases; 1/8 folded into k).
  * geometry wg:
      - dx/dy: ln fields via ACT Square(bias=-c)/Ln + DVE sub/clamp; phase
        fractions t = (alpha_j/4pi)*dx2 by f32r selector matmuls on PE;
        magic-round fold on DVE (rr, ff) + |f|-1/4 on Pool; ONE stacked ACT
        Sin pass yields [sin(2pi f); -cos(2pi f)] with the cos sign folded
        into the WBLK weights; WG contraction on PE (bf16).
      - dw/dh: exactly separable -> rank-64 PE contraction of per-box
        sin/cos banks.
      - h-major -> m-major partition permutation via 8 merged strided DMAs
        per row-block (3-dim APs).
  * exp-domain softmax: T = E*(1 + obj*wgd); E on ACT, obj-mult on Pool,
    wgd-mult/add on DVE; row sums via PE one-hot matmul; 1/s broadcast via
    PE selector matmul (f32r); final linear on PE.
  * bv is folded into bo on the host (bo' = bo + bv @ Wo).
"""
import math
import numpy as np
from contextlib import ExitStack

import concourse.bass as bass
import concourse.mybir as mybir
import concourse.tile as tile
from concourse.bass_utils import run_bass_kernel_spmd

F32 = mybir.dt.float32
F32R = mybir.dt.float32r
BF16 = mybir.dt.bfloat16
AF = mybir.ActivationFunctionType
ALU = mybir.AluOpType

B, N, D, H = 8, 512, 512, 8
DK = D // H
P = 128
NRB = N // P
NG = 8
GM = 16
WAVE_LEN = 1000.0
MAGIC = 12582912.0
C2 = float(2.0 * math.log(0.001))
ESHIFT = -6.0
TWO_PI = float(2.0 * math.pi)

_alphas = (100.0 / (WAVE_LEN ** (np.arange(8) / 8.0))).astype(np.float64)

# const-blob column offsets (f32 blob)
OFF_IDENT = 0          # [P, 128]
OFF_OH8 = 128          # [P, 64]  col = h*8 + c
OFF_WBLK = 192         # [P, 4*128]
OFF_W1E = 704          # [64, 128]
OFF_BG = 832           # [P, 1]
OFF_ACOL = 833         # [64, 1]
OFF_PCOLM = 834        # [64, 1]
OFF_PCOLN = 835        # [64, 1]
OFF_REPL = 836         # [8, 4*128]  col = ob*128 + p; 1 iff h == ob*2+p//64
CBLOB_W = 1348
# f32r blob: selap [P, 4*128] (col q*128+c)
CBLOBR_W = 512


def _split_multi_waits(nc):
    """walrus here accepts only ONE sync-wait per ISA instruction; hoist
    extras onto NoOps inserted before the offending instruction."""
    n_fix = 0
    for blk in nc.main_func.blocks:
        insts = list(blk.instructions)
        out, dirty = [], False
        for inst in insts:
            si = inst.sync_info
            waits = list(si.on_wait) if si is not None else []
            if len(waits) > 1:
                for kk, w in enumerate(waits[:-1]):
                    out.append(mybir.InstNoOp(
                        name=f"I-waitfix-{n_fix}-{kk}", engine=inst.engine,
                        sync_info=mybir.SyncInfo(on_wait=[w], on_update=[])))
                inst.sync_info = mybir.SyncInfo(
                    on_wait=[waits[-1]], on_update=list(si.on_update))
                n_fix += 1
                dirty = True
            out.append(inst)
        if dirty:
            blk.instructions = out
    return n_fix


def _build_cblob(WG, bG):
    cb = np.zeros((P, CBLOB_W), dtype=np.float32)
    cb[:, OFF_IDENT:OFF_IDENT + P] = np.eye(P, dtype=np.float32)
    # one-hot columns for row sums: OH8[p, h*8+c] = 1 iff c == h
    for h in range(H):
        cb[:, OFF_OH8 + h * H + h] = 1.0
    # WBLK: direct sin/cos weights.  sin4 layout: [:,0,:]=sin dx,
    # [:,1,:]=sin dy, [:,2,:]=cos dx, [:,3,:]=cos dy.
    gmap = [lambda j: j, lambda j: 8 + j, lambda j: 32 + j, lambda j: 40 + j]
    gscl = [1.0, 1.0, 1.0, 1.0]
    for c in range(4):
        for ml in range(GM):
            for j in range(8):
                for h in range(H):
                    cb[ml * 8 + j, OFF_WBLK + c * P + h * GM + ml] = \
                        gscl[c] * WG[h, gmap[c](j)]
    # dw/dh rank-64 weights (angle-addition banks; unchanged from the
    # half-angle formulation since the banks encode sin/cos via phase
    # offsets in PCOL)
    w1 = np.zeros((64, H), np.float32)
    acol = np.zeros((64,), np.float32)
    pcol_m = np.zeros((64,), np.float32)
    pcol_n = np.zeros((64,), np.float32)
    for f in range(2):
        for j in range(8):
            gs = 16 + 8 * f + j
            gc = 48 + 8 * f + j
            a = _alphas[j] / (4.0 * math.pi)
            for t in range(4):
                k = (f * 8 + j) * 4 + t
                acol[k] = a
                pcol_m[k] = 0.25 if t in (0, 2) else 0.0
                if t == 0:
                    pcol_n[k] = 0.0; w1[k] = WG[:, gs]
                elif t == 1:
                    pcol_n[k] = 0.75; w1[k] = WG[:, gs]   # -cos -> +pi
                elif t == 2:
                    pcol_n[k] = 0.25; w1[k] = WG[:, gc]
                else:
                    pcol_n[k] = 0.0; w1[k] = WG[:, gc]
    cb[0:64, OFF_W1E:OFF_W1E + P] = np.repeat(w1, GM, axis=1)
    cb[:, OFF_BG] = np.repeat(bG.astype(np.float64), GM).astype(np.float32)
    cb[0:64, OFF_ACOL] = acol
    cb[0:64, OFF_PCOLM] = pcol_m
    cb[0:64, OFF_PCOLN] = pcol_n
    for ob in range(H // 2):
        for hi in range(2):
            cb[ob * 2 + hi, OFF_REPL + ob * P + hi * DK:
               OFF_REPL + ob * P + (hi + 1) * DK] = 1.0
    return cb


def _build_cblobr():
    cr = np.zeros((P, CBLOBR_W), dtype=np.float32)
    # SELAP[64*W + q*16 + ml, q*128 + ml*8 + j] = alpha_j/(4pi)
    for W in range(2):
        for q in range(4):
            for ml in range(GM):
                for j in range(8):
                    cr[64 * W + q * GM + ml, q * P + ml * 8 + j] = \
                        _alphas[j] / (4.0 * math.pi)
    return cr


def _host_prep(inputs):
    q = np.asarray(inputs["input_query"], np.float32)
    k = np.asarray(inputs["input_key"], np.float32)
    v = np.asarray(inputs["input_value"], np.float32)
    box = np.asarray(inputs["input_box"], np.float32)
    mask = np.asarray(inputs["mask"])
    nobj = np.asarray(inputs["not_objects"])
    WG = np.asarray(inputs["WG"], np.float32)
    bG = np.asarray(inputs["bG"], np.float32)
    Wo = np.asarray(inputs["Wo"], np.float32)
    bo = np.asarray(inputs["bo"], np.float32)
    bv = np.asarray(inputs["bv"], np.float32)

    x_min, y_min, x_max, y_max = [box[..., i] for i in range(4)]
    cx = (x_min + x_max) * 0.5
    cy = (y_min + y_max) * 0.5
    ww = x_max - x_min + 1.0
    hh = y_max - y_min + 1.0
    l2w = (2.0 * np.log(ww)).astype(np.float32)
    l2h = (2.0 * np.log(hh)).astype(np.float32)

    maskcol = (np.where(mask == 0, -1e9, 0.0) + ESHIFT).astype(np.float32)
    obj = (1.0 - nobj.astype(np.float32)).astype(np.float32)
    borow = (bo.astype(np.float64) + bv.astype(np.float64)
             @ Wo.astype(np.float64)).astype(np.float32)

    def col(a):  # [N] -> [P, NRB]
        return a.reshape(NRB, P).T

    shared = {
        "Wq": np.asarray(inputs["Wq"], np.float32),
        "Wk": np.asarray(inputs["Wk"], np.float32),
        "Wv": np.asarray(inputs["Wv"], np.float32),
        "Wo": Wo,
        "CBLOB": _build_cblob(WG, bG),
        "CBLOBR": _build_cblobr(),
    }
    bqc = col(np.asarray(inputs["bq"], np.float32))
    bkc = col(np.asarray(inputs["bk"], np.float32))
    in_maps = []
    for b in range(B):
        cols = np.zeros((P, 28), np.float32)
        for ob in range(4):
            cols[:, 24 + ob] = 1.0
            cols[2 * ob, 24 + ob] = 0.0
            cols[2 * ob + 1, 24 + ob] = 0.0
        cols[:, 0:4] = col(maskcol[b])
        cols[:, 4:8] = bqc
        cols[:, 8:12] = bkc
        cols[:, 12:16] = -col(cx[b])
        cols[:, 16:20] = -col(cy[b])
        cols[:, 20:24] = col(obj[b])
        rows = np.stack([cx[b], cy[b], l2w[b], l2h[b], obj[b], borow], 0)
        m = dict(shared)
        m.update({
            "xq": q[b].copy(), "xk": k[b].copy(), "xv": v[b].copy(),
            "COLS": cols, "ROWS": rows.astype(np.float32).copy(),
        })
        in_maps.append(m)
    return in_maps


def build_nc(fix_waits=True, perm_merge=True, debug=False):
    nc = bass.Bass()

    def dp(name, shape, dt=F32):
        return nc.declare_dram_parameter(name, list(shape), dt, isOutput=False)

    xq = dp("xq", (N, D)); xk = dp("xk", (N, D)); xv = dp("xv", (N, D))
    Wq = dp("Wq", (D, D)); Wk = dp("Wk", (D, D)); Wv = dp("Wv", (D, D))
    Wo = dp("Wo", (D, D))
    CBLOB = dp("CBLOB", (P, CBLOB_W))
    CBLOBR = dp("CBLOBR", (P, CBLOBR_W), F32R)
    COLS = dp("COLS", (P, 28))
    ROWS = dp("ROWS", (6, N))
    out = nc.declare_dram_parameter("out", [N, D], F32, isOutput=True)
    wgd_dram = nc.dram_tensor("wgd_scratch", [NRB, H, P, N], BF16)

    with ExitStack() as ctx:
        tc = ctx.enter_context(tile.TileContext(nc))
        const = ctx.enter_context(tc.tile_pool(name="const", bufs=1))
        persist = ctx.enter_context(tc.tile_pool(name="persist", bufs=1))

        # ---- const loads (DMA order favors phase-1/2 start) ----
        loadp = ctx.enter_context(tc.tile_pool(name="loadp", bufs=1))
        xq_sb = loadp.tile([P, NRB, D], F32, tag="xq_sb")
        nc.sync.dma_start(xq_sb[:], xq.rearrange("(rb p) d -> p rb d", p=P))
        cF = const.tile([P, CBLOB_W], F32, tag="cF")
        # ident first: unblocks the PE transposes ~4us earlier than the
        # full blob would
        nc.sync.dma_start(cF[:, 0:P], CBLOB[:, 0:P])
        cols_t = const.tile([P, 28], F32, tag="cols")
        nc.sync.dma_start(cols_t[:], COLS[:])
        rows_t = const.tile([P, 6, N], F32, tag="rows")
        nc.sync.dma_start(rows_t[:, 0:4, :],
                          ROWS[None, 0:4, :].to_broadcast((P, 4, N)))
        nc.sync.dma_start(cF[:, P:], CBLOB[:, P:])
        nc.sync.dma_start(rows_t[:, 4:6, :],
                          ROWS[None, 4:6, :].to_broadcast((P, 2, N)))
        wq_f = loadp.tile([P, NRB, D], F32, tag="wq_f")
        nc.sync.dma_start(wq_f[:], Wq.rearrange("(kb p) d -> p kb d", p=P))
        xk_sb = loadp.tile([P, NRB, D], F32, tag="xk_sb")
        nc.sync.dma_start(xk_sb[:], xk.rearrange("(rb p) d -> p rb d", p=P))
        xv_sb = loadp.tile([P, NRB, D], F32, tag="xv_sb")
        nc.sync.dma_start(xv_sb[:], xv.rearrange("(rb p) d -> p rb d", p=P))
        wk_f = loadp.tile([P, NRB, D], F32, tag="wk_f")
        nc.sync.dma_start(wk_f[:], Wk.rearrange("(kb p) d -> p kb d", p=P))
        wv_f = loadp.tile([P, NRB, D], F32, tag="wv_f")
        nc.sync.dma_start(wv_f[:], Wv.rearrange("(kb p) d -> p kb d", p=P))
        cR = const.tile([P, CBLOBR_W], F32R, tag="cR")
        nc.sync.dma_start(cR[:], CBLOBR[:])
        wo_f = loadp.tile([P, NRB, D], F32, tag="wo_f")
        nc.sync.dma_start(wo_f[:], Wo.rearrange("(kb p) d -> p kb d", p=P))

        ident = cF[:, OFF_IDENT:OFF_IDENT + P]
        mcol = cols_t[:, 0:4]
        bqcol = cols_t[:, 4:8]
        bkcol = cols_t[:, 8:12]
        negcx = cols_t[:, 12:16]
        negcy = cols_t[:, 16:20]
        ocol = cols_t[:, 20:24]
        zcol = cols_t[:, 24:28]
        cxbc = rows_t[:, 0, :]
        cybc = rows_t[:, 1, :]
        l2wbc = rows_t[:, 2, :]
        l2hbc = rows_t[:, 3, :]
        objbc_f = rows_t[:, 4, :]
        bobc = rows_t[:, 5, :]

        # small const casts / derived
        oh8_b = const.tile([P, H * H], BF16, tag="oh8b")
        nc.vector.tensor_copy(oh8_b[:], cF[:, OFF_OH8:OFF_OH8 + H * H])
        wblk_b = const.tile([P, 4, P], BF16, tag="wblkb")
        for c in range(4):
            nc.gpsimd.tensor_copy(wblk_b[:, c, :],
                                  cF[:, OFF_WBLK + c * P:OFF_WBLK + (c + 1) * P])
        w1e_b = const.tile([64, P], BF16, tag="w1eb")
        nc.gpsimd.tensor_copy(w1e_b[:], cF[0:64, OFF_W1E:OFF_W1E + P])
        objbc = const.tile([P, N], BF16, tag="objbc")
        nc.gpsimd.tensor_copy(objbc[:], objbc_f[:])
        halfpi = const.tile([P, 1], F32, tag="halfpi")
        nc.vector.memset(halfpi[:], float(math.pi / 2.0))
        bgm1 = const.tile([P, 1], F32, tag="bgm1")
        nc.vector.tensor_scalar(bgm1[:], cF[:, OFF_BG:OFF_BG + 1], -1.0, None,
                                ALU.add)
        acol = cF[0:64, OFF_ACOL:OFF_ACOL + 1]
        pcolm = cF[0:64, OFF_PCOLM:OFF_PCOLM + 1]
        pcoln = cF[0:64, OFF_PCOLN:OFF_PCOLN + 1]

        # ---------------- phases 1+2 (shared scope so they overlap) -------
        dxy2 = persist.tile([P, NRB, 2, N], F32R, tag="dxy2")
        bankM = persist.tile([64, N], BF16, tag="bankM")
        bankN = persist.tile([64, N], BF16, tag="bankN")
        qT = persist.tile([P, NRB, N], BF16, tag="qT")
        kTt = persist.tile([P, NRB, N], BF16, tag="kT")
        v_sb = persist.tile([P, NRB, D], BF16, tag="v_sb")
        wo_b = persist.tile([P, NRB, D], BF16, tag="wob")
        ot = persist.tile([P, NRB, N], BF16, tag="ot")

        with tc.tile_pool(name="tpool", bufs=1) as tpool, \
             tc.tile_pool(name="work2", bufs=3) as work2, \
             tc.tile_pool(name="work3", bufs=1) as work3, \
             tc.tile_pool(name="work1", bufs=2) as work1, \
             tc.tile_pool(name="psum1", bufs=4, space="PSUM") as psum1:
            # phase 2: ln fields (ACT Square/Ln + DVE sub/clamp)
            for rb in range(NRB):
                for (ci, cbc, ncol, l2bc) in ((0, cxbc, negcx, l2wbc),
                                              (1, cybc, negcy, l2hbc)):
                    d2 = work2.tile([P, N], F32, tag="geo_d2")
                    nc.scalar.activation(d2[:], cbc, AF.Square,
                                         bias=ncol[:, rb:rb + 1])
                    l2t = work2.tile([P, N], F32, tag="geo_l2")
                    nc.scalar.activation(l2t[:], d2[:], AF.Ln)
                    g_ = work2.tile([P, N], F32, tag="geo_g")
                    nc.vector.tensor_tensor(g_[:], l2t[:], l2bc, ALU.subtract)
                    nc.vector.tensor_scalar_max(dxy2[:, rb, ci, :], g_[:], C2)

            # phase 3: dw/dh banks (early; DVE idle at start)
            for (pcol, bank) in ((pcolm, bankM), (pcoln, bankN)):
                t_ = work3.tile([64, N], F32, tag="bk_t")
                nc.vector.tensor_scalar(t_[:32, :], l2wbc[:32, :],
                                        acol[:32, :], pcol[:32, :],
                                        ALU.mult, ALU.add)
                nc.vector.tensor_scalar(t_[32:, :], l2hbc[32:64, :],
                                        acol[32:, :], pcol[32:, :],
                                        ALU.mult, ALU.add)
                r_ = work3.tile([64, N], F32, tag="bk_r")
                nc.vector.tensor_scalar(r_[:], t_[:], MAGIC, -MAGIC,
                                        ALU.add, ALU.add)
                f_ = work3.tile([64, N], F32, tag="bk_f")
                nc.vector.tensor_tensor(f_[:], t_[:], r_[:], ALU.subtract)
                nc.scalar.activation(bank[:], f_[:], AF.Sin, scale=TWO_PI)

            # phase 1: transposes + projections
            xqTb = tpool.tile([P, NRB, N], BF16, tag="xqTb")
            xkTb = tpool.tile([P, NRB, N], BF16, tag="xkTb")
            xvTb = tpool.tile([P, NRB, N], BF16, tag="xvTb")
            wq_b = tpool.tile([P, NRB, D], BF16, tag="wqb")
            wk_b = tpool.tile([P, NRB, D], BF16, tag="wkb")
            wv_b = tpool.tile([P, NRB, D], BF16, tag="wvb")

            kk = 0
            for (xs, dstb) in ((xq_sb, xqTb), (xk_sb, xkTb), (xv_sb, xvTb)):
                for rb in range(NRB):
                    for cb in range(NRB):
                        tp = psum1.tile([P, P], F32, tag="tp")
                        nc.tensor.transpose(tp[:], xs[:, rb, cb * P:(cb + 1) * P],
                                            ident)
                        dst = dstb[:, cb, rb * P:(rb + 1) * P]
                        if kk % 2 == 0:
                            nc.vector.tensor_copy(dst, tp[:])
                        else:
                            nc.scalar.activation(dst, tp[:], AF.Identity)
                        kk += 1
            for (wf, wb_) in ((wq_f, wq_b), (wk_f, wk_b), (wv_f, wv_b),
                              (wo_f, wo_b)):
                nc.gpsimd.tensor_copy(wb_[:], wf[:])

            for (wb_, xb, dstT, bcol, scl) in (
                    (wq_b, xqTb, qT, bqcol, 1.0),
                    (wk_b, xkTb, kTt, bkcol, 0.125)):
                for ob in range(NRB):
                    ps = psum1.tile([P, N], F32, tag="projps")
                    for kb in range(NRB):
                        nc.tensor.matmul(ps[:],
                                         wb_[:, kb, ob * P:(ob + 1) * P],
                                         xb[:, kb, :],
                                         start=(kb == 0),
                                         stop=(kb == NRB - 1))
                    nc.scalar.activation(dstT[:, ob, :], ps[:], AF.Identity,
                                         scale=scl, bias=bcol[:, ob:ob + 1])
            for mb in range(NRB):
                ps = psum1.tile([P, D], F32, tag="projps")
                for kb in range(NRB):
                    nc.tensor.matmul(ps[:], xvTb[:, kb, mb * P:(mb + 1) * P],
                                     wv_b[:, kb, :],
                                     start=(kb == 0), stop=(kb == NRB - 1))
                nc.scalar.activation(v_sb[:, mb, :], ps[:], AF.Identity)

        # ---------------- phase 4: wg ----------------
        wgdT = persist.tile([P, H, NRB, N], BF16, tag="wgdT")
        with tc.tile_pool(name="work4", bufs=2) as work4, \
             tc.tile_pool(name="psum_u", bufs=2, space="PSUM") as psum_u, \
             tc.tile_pool(name="psum_wg", bufs=3, space="PSUM") as psum_wg:
            for rb in range(NRB):
                wgd_il = work4.tile([P, NG, N], BF16, tag="wgd_il")
                for g in range(NG):
                    off = 64 * (g // 4)
                    qq = g % 4
                    ups = psum_u.tile([P, 2, N], F32, tag="ups")
                    for ci in range(2):
                        nc.tensor.matmul(ups[:, ci, :],
                                         cR[off:off + 64, qq * P:(qq + 1) * P],
                                         dxy2[off:off + 64, rb, ci, :],
                                         start=True, stop=True)
                    rr = work4.tile([P, 2, N], F32, tag="fold_r")
                    nc.vector.tensor_scalar(rr[:], ups[:], MAGIC, -MAGIC,
                                            ALU.add, ALU.add)
                    ff = work4.tile([P, 2, N], F32, tag="fold_f")
                    nc.vector.tensor_tensor(ff[:], ups[:], rr[:],
                                            ALU.subtract)
                    habs = work4.tile([P, 2, N], F32, tag="habs")
                    nc.scalar.activation(habs[:], ff[:], AF.Abs)
                    sin4 = work4.tile([P, 4, N], BF16, tag="sin4")
                    nc.scalar.activation(sin4[:, 0:2, :], ff[:], AF.Sin,
                                         scale=TWO_PI)
                    # cos(2pi f) = sin(pi/2 - 2pi |f|)
                    nc.scalar.activation(sin4[:, 2:4, :], habs[:], AF.Sin,
                                         scale=-TWO_PI, bias=halfpi[:])
                    lhs_wh = work4.tile([64, P], BF16, tag="lhs_wh")
                    mbase = rb * P + g * GM
                    nc.gpsimd.tensor_tensor(
                        lhs_wh[:].rearrange("k (h m) -> k h m", h=H),
                        w1e_b[:].rearrange("k (h m) -> k h m", h=H),
                        bankM[:, mbase:mbase + GM][:, None, :]
                            .to_broadcast((64, H, GM)),
                        ALU.mult)
                    wgp = psum_wg.tile([P, N], F32, tag="wgp")
                    for c in range(4):
                        nc.tensor.matmul(wgp[:], wblk_b[:, c, :],
                                         sin4[:, c, :],
                                         start=(c == 0), stop=False)
                    nc.tensor.matmul(wgp[:], lhs_wh[:], bankN[:],
                                     start=False, stop=True)
                    # wgd = max(wg + bG, 1e-6) - 1 = max(wg + (bG-1), 1e-6-1)
                    nc.vector.tensor_scalar(wgd_il[:, g, :], wgp[:],
                                            bgm1[:], 1e-6 - 1.0,
                                            ALU.add, ALU.max)
                # h-major -> m-major permutation via DRAM bounce
                # (SBUF->SBUF DMA honors only one partition dim on HW, and
                # SBUF-side APs may carry only one partition dim, so the
                # write side goes per (rb, h)).
                for h in range(H):
                    nc.sync.dma_start(
                        wgd_dram[rb, h]
                            .rearrange("(g ml) n -> ml g n", g=NG),
                        wgd_il[h * GM:(h + 1) * GM, :, :])
                nc.sync.dma_start(
                    wgdT[:, :, rb, :],
                    wgd_dram[rb].rearrange("h p n -> p h n"))

        # ---------------- phase 5: attention ----------------
        with tc.tile_pool(name="work5", bufs=3) as work5, \
             tc.tile_pool(name="psum5", bufs=2, space="PSUM") as psum5, \
             tc.tile_pool(name="psum_s", bufs=1, space="PSUM") as psum_s, \
             tc.tile_pool(name="psum_av", bufs=2, space="PSUM") as psum_av, \
             tc.tile_pool(name="psum_rb", bufs=1, space="PSUM") as psum_rb, \
             tc.tile_pool(name="dbgpool", bufs=1) as dbgpool:

            objpair = persist.tile([P, NRB, N], BF16, tag="objpair")
            for rb in range(NRB):
                nc.vector.tensor_scalar(objpair[:, rb, :], objbc[:],
                                        ocol[:, rb:rb + 1], None, ALU.mult)
            # head PAIRS (2k, 2k+1) share kT/qT block ob=k at offsets 0/64
            for ob in range(H // 2):
                h0 = 2 * ob
                av = psum_av.tile([P, N], F32, tag="avps")
                sbank = psum_s.tile([H, N], F32, tag="sbank")
                for rb in range(NRB):
                    st2 = psum5.tile([P, 2, N], F32, tag="stps")
                    for hi in range(2):
                        po = hi * DK
                        nc.tensor.matmul(
                            st2[:, hi, :],
                            kTt[po:po + DK, ob, rb * P:(rb + 1) * P],
                            qT[po:po + DK, ob, :], start=True, stop=True)
                    e_ = work5.tile([P, 2, N], BF16, tag="e_t")
                    nc.scalar.activation(e_[:], st2[:], AF.Exp,
                                         bias=mcol[:, rb:rb + 1])
                    e1 = work5.tile([P, 2, N], BF16, tag="e1_t")
                    e1_eng = nc.gpsimd if (ob + rb) % 2 == 0 else nc.vector
                    e1_eng.tensor_tensor(
                        e1[:], e_[:],
                        objpair[:, rb, None, :].to_broadcast((P, 2, N)),
                        ALU.mult)
                    e2 = work5.tile([P, 2, N], BF16, tag="e2_t")
                    nc.vector.tensor_tensor(e2[:], e1[:],
                                            wgdT[:, h0:h0 + 2, rb, :],
                                            ALU.mult)
                    tt_ = work5.tile([P, 2, N], BF16, tag="tt_t")
                    nc.vector.tensor_tensor(tt_[:], e_[:], e2[:], ALU.add)
                    for hi in range(2):
                        po = hi * DK
                        nc.tensor.matmul(sbank[:],
                                         oh8_b[:, (h0 + hi) * H:
                                               (h0 + hi + 1) * H],
                                         tt_[:, hi, :],
                                         start=(rb == 0 and hi == 0),
                                         stop=(rb == NRB - 1 and hi == 1),
                                         skip_group_check=True)
                        nc.tensor.matmul(av[po:po + DK, :],
                                         v_sb[:, rb,
                                              (h0 + hi) * DK:(h0 + hi + 1) * DK],
                                         tt_[:, hi, :], start=(rb == 0),
                                         stop=(rb == NRB - 1),
                                         skip_group_check=True)
                if debug and ob == 0:
                    dbg_sb = nc.declare_dram_parameter(
                        "dbg_sbank", [H, N], F32, isOutput=True)
                    sb_c = dbgpool.tile([H, N], F32, tag="dbg_sbc")
                    nc.vector.tensor_copy(sb_c[:], sbank[:])
                    nc.sync.dma_start(dbg_sb[:], sb_c[:])
                    dbg_av = nc.declare_dram_parameter(
                        "dbg_av", [P, N], F32, isOutput=True)
                    av_c = dbgpool.tile([P, N], F32, tag="dbg_avc")
                    nc.vector.tensor_copy(av_c[:], av[:])
                    nc.sync.dma_start(dbg_av[:], av_c[:])
                sb2 = work5.tile([H, N], F32, tag="sb2")
                nc.vector.tensor_scalar(sb2[:], sbank[:],
                                        zcol[0:8, ob:ob + 1], None, ALU.add)
                rs = work5.tile([H, N], F32, tag="rs")
                nc.vector.reciprocal(rs[:], sb2[:])
                rrb = psum_rb.tile([P, N], F32, tag="rrb")
                nc.tensor.matmul(rrb[:], cF[0:8, OFF_REPL + ob * P:
                                            OFF_REPL + (ob + 1) * P],
                                 rs[0:8, :], start=True, stop=True)
                if debug and ob == 0:
                    dbg_rs = nc.declare_dram_parameter(
                        "dbg_rs", [H, N], F32, isOutput=True)
                    nc.sync.dma_start(dbg_rs[:], rs[:])
                    dbg_rrb = nc.declare_dram_parameter(
                        "dbg_rrb", [P, N], F32, isOutput=True)
                    rrb_c = dbgpool.tile([P, N], F32, tag="dbg_rrbc")
                    nc.vector.tensor_copy(rrb_c[:], rrb[:])
                    nc.sync.dma_start(dbg_rrb[:], rrb_c[:])
                rrb_sb = work5.tile([P, N], F32, tag="rrb_sb")
                nc.scalar.activation(rrb_sb[:], rrb[:], AF.Identity)
                nc.vector.tensor_tensor(ot[:, ob, :], av[:], rrb_sb[:],
                                        ALU.mult)

        if debug:
            dbg_dxy2 = nc.declare_dram_parameter(
                "dbg_dxy2", [P, NRB, 2, N], F32R, isOutput=True)
            nc.sync.dma_start(dbg_dxy2[:], dxy2[:])
            dbg_wgdT = nc.declare_dram_parameter(
                "dbg_wgdT", [P, H, NRB, N], BF16, isOutput=True)
            nc.sync.dma_start(dbg_wgdT[:], wgdT[:])
            dbg_qT = nc.declare_dram_parameter(
                "dbg_qT", [P, NRB, N], BF16, isOutput=True)
            nc.sync.dma_start(dbg_qT[:], qT[:])
            dbg_kT = nc.declare_dram_parameter(
                "dbg_kT", [P, NRB, N], BF16, isOutput=True)
            nc.sync.dma_start(dbg_kT[:], kTt[:])
            dbg_v = nc.declare_dram_parameter(
                "dbg_v", [P, NRB, D], BF16, isOutput=True)
            nc.sync.dma_start(dbg_v[:], v_sb[:])
            dbg_ot = nc.declare_dram_parameter(
                "dbg_ot", [P, NRB, N], BF16, isOutput=True)
            nc.sync.dma_start(dbg_ot[:], ot[:])

        # final projection: out[n, d]
        with tc.tile_pool(name="work6", bufs=3) as work6, \
             tc.tile_pool(name="psum6", bufs=4, space="PSUM") as psum6:
            for r in range(NRB):
                ps = psum6.tile([P, D], F32, tag="fps")
                for kt in range(NRB):
                    nc.tensor.matmul(ps[:], ot[:, kt, r * P:(r + 1) * P],
                                     wo_b[:, kt, :],
                                     start=(kt == 0), stop=(kt == NRB - 1))
                fo = work6.tile([P, D], F32, tag="fo")
                nc.vector.tensor_tensor(fo[:], ps[:], bobc, ALU.add)
                nc.sync.dma_start(out[r * P:(r + 1) * P, :], fo[:])

    if fix_waits:
        _split_multi_waits(nc)
    return nc


_NC_CACHE = {}


def kernel(**inputs):
    in_maps = _host_prep(inputs)
    if "nc" not in _NC_CACHE:
        _NC_CACHE["nc"] = build_nc()
    nc = _NC_CACHE["nc"]
    res = run_bass_kernel_spmd(nc, in_maps, list(range(B)))
    out = np.stack([res.results[b]["out"] for b in range(B)], axis=0)
    return out.astype(np.float32)


if __name__ == "__main__":
    print("kernel module ok")
